# revision 38
# baseline (speedup 1.0000x reference)
"""Trainium2 Bass kernel for nn_Block_46059229282655 (dense transformer block).

Sharding: sequence-parallel over 8 NeuronCores (512 tokens each), weights
replicated.  K/V for both attentions are AllGathered (bf16, packed).  All
activations are kept feature-major ([C_chunk=128 partitions, tokens free]) so
matmuls never need transposes; V is gathered token-major with a baked-in ones
column per head so the softmax denominator falls out of the PV matmul.
"""

import os
from contextlib import ExitStack

import numpy as np
import ml_dtypes

import concourse.bass as bass
import concourse.mybir as mybir
import concourse.tile as tile
from concourse import bacc
from concourse.bass_utils import run_bass_kernel_spmd

BFNP = ml_dtypes.bfloat16
F32 = mybir.dt.float32
BF16 = mybir.dt.bfloat16
F8 = mybir.dt.float8e4
I16 = mybir.dt.int16
I8 = mybir.dt.int8
DROW = mybir.MatmulPerfMode.DoubleRow
AF = mybir.ActivationFunctionType
ALU = mybir.AluOpType

# Schraudolph fast-exp constants: exp(x*s) ~= bitcast_bf16(int16(x*(s*log2e*128) + B))
LOG2E = 1.4426950408889634
SCHRAU_B = 127.0 * 128.0 - 8.8
SCHRAU_B8 = 7.0 * 8.0 - 0.55

R = 8            # cores
P = 128          # partitions
T = 512          # tokens per core
N = R * T        # 4096 tokens
C = 768
CC = C // P      # 6 channel chunks
NHS, HDS = 12, 64
NHC, HDC = 8, 96
FF = 3072
FC = FF // P     # 24
NKC = N // P     # 32 key chunks
WVS = NHS * (HDS + 1)   # 780
WVC = NHC * (HDC + 1)   # 776
WVP = 784               # padded V row (DoubleRow chunk stride must be %16)
KT_E = C * T            # 393216 elements of a K^T block
VS_E = T * WVP
VC_E = T * WVP
KVS = KT_E + VS_E
KVC = KT_E + VC_E
EPS = 1e-5

KDBG = bool(os.environ.get("KDBG"))

_BUILT = None


def _build():
    nc = bacc.Bacc(None, target_bir_lowering=False, debug=False)
    dt = mybir.dt

    # ---------------- I/O ----------------
    xT_d = nc.dram_tensor("xT", [C, T], F32, kind="ExternalInput")
    xTb_d = nc.dram_tensor("xTb", [C, T], BF16, kind="ExternalInput")
    yT_d = nc.dram_tensor("yT", [C, T], BF16, kind="ExternalInput")
    zT_d = nc.dram_tensor("zT", [C, T], BF16, kind="ExternalInput")

    w_q_d = nc.dram_tensor("w_q", [C, C], BF16, kind="ExternalInput")
    w_k_d = nc.dram_tensor("w_k", [C, C], F8, kind="ExternalInput")
    w_ve_d = nc.dram_tensor("w_ve", [C, WVS], F8, kind="ExternalInput")
    vb_e_d = nc.dram_tensor("vb_e", [1, WVS], BF16, kind="ExternalInput")
    vgwn_d = nc.dram_tensor("vgwn", [1, WVS], BF16, kind="ExternalInput")
    cstv_d = nc.dram_tensor("cstv", [1, WVS], BF16, kind="ExternalInput")
    kgwn_d = nc.dram_tensor("kgwn", [1, C], BF16, kind="ExternalInput")
    kcst_d = nc.dram_tensor("kcst", [1, C], BF16, kind="ExternalInput")
    qgwn_d = nc.dram_tensor("qgwn", [1, C], BF16, kind="ExternalInput")
    qcst_d = nc.dram_tensor("qcst", [1, C], BF16, kind="ExternalInput")
    sqgwn_d = nc.dram_tensor("sqgwn", [1, C], BF16, kind="ExternalInput")
    sqcst_d = nc.dram_tensor("sqcst", [1, C], BF16, kind="ExternalInput")
    w_proj_d = nc.dram_tensor("w_proj", [C, C], BF16, kind="ExternalInput")
    w_caq_d = nc.dram_tensor("w_caq", [C, C], BF16, kind="ExternalInput")
    w_cak_d = nc.dram_tensor("w_cak", [C, C], BF16, kind="ExternalInput")
    w_cave_d = nc.dram_tensor("w_cave", [C, WVC], BF16, kind="ExternalInput")
    vbc_e_d = nc.dram_tensor("vbc_e", [1, WVC], BF16, kind="ExternalInput")
    w_cao_d = nc.dram_tensor("w_cao", [C, C], BF16, kind="ExternalInput")
    w_m2a_d = nc.dram_tensor("w_m2a", [C, FF], BF16, kind="ExternalInput")
    w_m2b_d = nc.dram_tensor("w_m2b", [FF, C], BF16, kind="ExternalInput")
    w_m1a_d = nc.dram_tensor("w_m1a", [C, FF], BF16, kind="ExternalInput")
    w_m1b_d = nc.dram_tensor("w_m1b", [FF, C], BF16, kind="ExternalInput")
    w_pw1_d = nc.dram_tensor("w_pw1", [C, C], BF16, kind="ExternalInput")
    w_pw2_d = nc.dram_tensor("w_pw2", [C, C], BF16, kind="ExternalInput")

    # [parts, k] fp32 vectors (host pre-reshaped (k,parts)->T)
    vec_specs = {
        "ln1_g": (P, CC), "ln1_b": (P, CC), "ln2_g": (P, CC), "ln2_b": (P, CC),
        "qb": (P, CC), "kb": (P, CC), "kgw": (P, CC), "projb": (P, CC),
        "caob": (P, CC),
        "m2b1": (P, FC), "m2b2": (P, CC), "m1b1": (P, FC), "m1b2": (P, CC),
        "pw1b": (P, CC), "pw2b": (P, CC),
    }
    vec_d = {k: nc.dram_tensor(k, list(s), F32, kind="ExternalInput")
             for k, s in vec_specs.items()}

    o_p1 = nc.dram_tensor("o_p1", [C, T], F32, kind="ExternalOutput")
    o_pw1 = nc.dram_tensor("o_pw1", [C, T], F32, kind="ExternalOutput")
    o_pw2 = nc.dram_tensor("o_pw2", [C, T], F32, kind="ExternalOutput")

    kvKV_in = nc.dram_tensor("kvKV_in", [KVS], F8)
    kvKV_out = nc.dram_tensor("kvKV_out", [R, KVS], F8, addr_space="Shared")
    rs_scr = nc.dram_tensor("rs_scr", [T], F32)
    kvC_in = nc.dram_tensor("kvC_in", [KVC], F8)
    kvC_out = nc.dram_tensor("kvC_out", [R, KVC], F8, addr_space="Shared")

    dbg = {}
    if KDBG:
        for nm in ("d_x1", "d_x1f", "d_x2", "d_p2", "d_at", "d_h1"):
            dbg[nm] = nc.dram_tensor(nm, [C, T], F32, kind="ExternalOutput")

    with tile.TileContext(nc) as tc, ExitStack() as top:
        # ------------- global pools -------------
        cpool = top.enter_context(tc.tile_pool(name="consts", bufs=1))
        statp = top.enter_context(tc.tile_pool(name="statp", bufs=1))
        lnp = top.enter_context(tc.tile_pool(name="lnp", bufs=1))
        w66p = top.enter_context(tc.tile_pool(name="w66p", bufs=1))
        ps_st = top.enter_context(tc.tile_pool(name="ps_st", bufs=1, space="PSUM"))
        ps_ot = top.enter_context(tc.tile_pool(name="ps_ot", bufs=1, space="PSUM"))
        ps_mm = top.enter_context(tc.tile_pool(name="ps_mm", bufs=1, space="PSUM"))

        # ------------- constants -------------
        vec_specs["qb"] = (P, CC)
        vcc_names = [k for k, s in vec_specs.items() if s == (P, CC)]
        vfc_names = [k for k, s in vec_specs.items() if s == (P, FC)]
        vcc_t = cpool.tile([P, len(vcc_names) * CC], F32, tag="vcc", name="vcc_t")
        vfc_t = cpool.tile([P, len(vfc_names) * FC], F32, tag="vfc", name="vfc_t")
        vec = {}
        for i, k in enumerate(vcc_names):
            nc.gpsimd.dma_start(vcc_t[:, i * CC:(i + 1) * CC], vec_d[k][:])
            vec[k] = vcc_t[:, i * CC:(i + 1) * CC]
        for i, k in enumerate(vfc_names):
            nc.gpsimd.dma_start(vfc_t[:, i * FC:(i + 1) * FC], vec_d[k][:])
            vec[k] = vfc_t[:, i * FC:(i + 1) * FC]
        vbc_sb = cpool.tile([1, WVC], BF16, tag="vbc", name="vbc_sb")
        nc.gpsimd.dma_start(vbc_sb[:], vbc_e_d[:])
        ones_col = cpool.tile([P, 1], BF16, tag="oc", name="ones_col")
        nc.vector.memset(ones_col[:], 1.0)
        ones_row = cpool.tile([1, P], BF16, tag="or", name="ones_row")
        nc.vector.memset(ones_row[:], 1.0)
        eps_t = cpool.tile([1, 1], F32, tag="eps", name="eps_t")
        nc.vector.memset(eps_t[:], float(EPS))
        # PE warmup: sustain HAM busy-window from t~1us so phase-A GEMMs run
        # at 2.4GHz instead of the 1.2GHz cold clock
        warm_t = cpool.tile([1, T], BF16, tag="wt", name="warm_t")
        nc.vector.memset(warm_t[:], 1.0)
        for i in range(48):
            wp = ps_mm.tile([1, T], F32, tag="mm", bufs=2, name=f"warm{i}")
            nc.tensor.matmul(wp[:], ones_col[0:1, 0:1], warm_t[0:1, 0:T],
                             start=True, stop=True)
        _wf = [0]

        def warm_fill(n):
            # dependency-free PE work to bridge sparse stretches so the HAM
            # clock gate never sees an idle MID window (it is slow to re-warm)
            for _ in range(n):
                i = _wf[0]
                _wf[0] += 1
                wp = ps_mm.tile([1, T], F32, tag="mm", bufs=2, name=f"wf{i}")
                nc.tensor.matmul(wp[:], ones_col[0:1, 0:1], warm_t[0:1, 0:T],
                                 start=True, stop=True)

        vgwn_r = cpool.tile([1, WVS], BF16, tag="vgr", name="vgwn_r")
        nc.gpsimd.dma_start(vgwn_r[:], vgwn_d[:])
        cstv_r = cpool.tile([1, WVS], BF16, tag="cvr", name="cstv_r")
        nc.gpsimd.dma_start(cstv_r[:], cstv_d[:])
        kgwn_r = cpool.tile([1, C], BF16, tag="kgr", name="kgwn_r")
        nc.gpsimd.dma_start(kgwn_r[:], kgwn_d[:])
        kcst_r = cpool.tile([1, C], BF16, tag="kcr", name="kcst_r")
        nc.gpsimd.dma_start(kcst_r[:], kcst_d[:])
        qgwn_r = cpool.tile([1, C], BF16, tag="qgr", name="qgwn_r")
        nc.gpsimd.dma_start(qgwn_r[:], qgwn_d[:])
        qcst_r = cpool.tile([1, C], BF16, tag="qcr", name="qcst_r")
        nc.gpsimd.dma_start(qcst_r[:], qcst_d[:])
        sqgwn_r = cpool.tile([1, C], BF16, tag="sqg", name="sqgwn_r")
        nc.gpsimd.dma_start(sqgwn_r[:], sqgwn_d[:])
        sqcst_r = cpool.tile([1, C], BF16, tag="sqc", name="sqcst_r")
        nc.gpsimd.dma_start(sqcst_r[:], sqcst_d[:])

        # ------------- helpers -------------
        def ln_stats(src, nm):
            """src: [P, CC, T] fp32 SBUF. Returns psum broadcasts (rstd_b, mrstd_b)."""
            if src.dtype == BF16:
                xb = src
            else:
                xb = lnp.tile([P, CC, T], BF16, tag="lnxb", bufs=1, name=f"xb_{nm}")
            sq = lnp.tile([P, CC, T], BF16, tag="lnsq", bufs=1, name=f"sq_{nm}")
            for c in range(CC):
                if xb is not src:
                    nc.vector.tensor_copy(xb[:, c], src[:, c])
                nc.vector.tensor_mul(sq[:, c], xb[:, c], xb[:, c])
            s1 = ps_mm.tile([1, T], F32, tag="mm", bufs=2, name=f"s1_{nm}")
            for c in range(CC):
                nc.tensor.matmul(s1[:], ones_col[:], xb[:, c],
                                 start=(c == 0), stop=(c == CC - 1))
            s2 = ps_mm.tile([1, T], F32, tag="mm", bufs=2, name=f"s2_{nm}")
            for c in range(CC):
                nc.tensor.matmul(s2[:], ones_col[:], sq[:, c],
                                 start=(c == 0), stop=(c == CC - 1))
            return ln_stats_from_sums(s1, s2, nm), xb

        stats_cells = {}

        def ln_stats_from_sums(s1, s2, nm):
            m = statp.tile([1, T], F32, tag="stat", bufs=4, name=f"m_{nm}")
            nc.vector.tensor_scalar(m[:], s1[:], 1.0 / C, None, ALU.mult)
            ex2 = statp.tile([1, T], F32, tag="stat", bufs=4, name=f"e2_{nm}")
            nc.vector.tensor_scalar(ex2[:], s2[:], 1.0 / C, None, ALU.mult)
            msq = statp.tile([1, T], F32, tag="stat", bufs=4, name=f"ms_{nm}")
            nc.vector.tensor_mul(msq[:], m[:], m[:])
            var = statp.tile([1, T], F32, tag="stat", bufs=4, name=f"va_{nm}")
            nc.vector.tensor_sub(var[:], ex2[:], msq[:])
            sd = statp.tile([1, T], F32, tag="stat", bufs=4, name=f"sd_{nm}")
            nc.scalar.activation(sd[:], var[:], AF.Sqrt, bias=eps_t[:])
            stats_cells[nm + ".msd"] = (m, sd)
            rstd = statp.tile([1, T], F32, tag="stat", bufs=4, name=f"rs_{nm}")
            nc.vector.reciprocal_approx_fast(rstd[:], sd[:])
            mr = statp.tile([1, T], F32, tag="stat", bufs=4, name=f"mr_{nm}")
            nc.vector.tensor_mul(mr[:], m[:], rstd[:])
            stats_cells[nm] = (rstd, mr)
            rstd_b = statp.tile([1, T], BF16, tag="statb", bufs=2, name=f"rb_{nm}")
            nc.vector.tensor_copy(rstd_b[:], rstd[:])
            stats_cells[nm + ".rb"] = rstd_b
            mr_b = statp.tile([1, T], BF16, tag="statb", bufs=2, name=f"mb_{nm}")
            nc.vector.tensor_copy(mr_b[:], mr[:])
            # broadcast to all partitions via PE, then evacuate to SBUF so no
            # PSUM bank stays pinned across the phase
            bc_sb = lnp.tile([P, 2, T], F32, tag="bcsb", bufs=2, name=f"bcs_{nm}")
            for i, v in enumerate((rstd_b, mr_b)):
                bp = ps_mm.tile([P, T], F32, tag="mm", bufs=2, name=f"bp_{nm}{i}")
                nc.tensor.matmul(bp[:], ones_row[:], v[:], start=True, stop=True)
                nc.vector.tensor_copy(bc_sb[:, i], bp[:])
            return bc_sb

        def ln_apply(src, bc, g, b, dst, nm, eng=None):
            """dst[:, c] = ((src*rstd) - m*rstd) * g + b, bf16 out."""
            e = eng or nc.vector
            for c in range(CC):
                u = lnp.tile([P, T], F32, tag="lnu", bufs=3, name=f"u_{nm}{c}")
                e.tensor_mul(u[:], src[:, c], bc[:, 0])
                e.tensor_sub(u[:], u[:], bc[:, 1])
                e.tensor_scalar(dst[:, c], u[:], g[:, c:c + 1], b[:, c:c + 1],
                                ALU.mult, ALU.add)

        def load_w66(dram, nm, pool=None, htag="w66", parts=P, hdim=CC,
                     eng=None, dtype=BF16):
            wp = pool or w66p
            ap = dram if isinstance(dram, bass.AP) else dram[:]
            wt = wp.tile([parts, hdim, ap.shape[-1]], dtype, tag=htag, bufs=2,
                         name=f"w_{nm}")
            (eng or nc.sync).dma_start(
                wt[:], ap.rearrange("(a p) n -> p a n", p=parts))
            return wt

        def linear_fm(dst, src, w_sb, bias, func, nm, cin=CC, dout=CC):
            """dst [P, dout, T] <- act(W^T @ src + bias); w_sb [P, cin, dout*128]."""
            for d in range(dout):
                ps = ps_mm.tile([P, T], F32, tag="mm", bufs=2, name=f"p_{nm}{d}")
                for c in range(cin):
                    nc.tensor.matmul(ps[:], w_sb[:, c, d * P:(d + 1) * P], src[:, c],
                                     start=(c == 0), stop=(c == cin - 1))
                if func is None:
                    nc.vector.tensor_copy(dst[:, d], ps[:])
                elif func is AF.Identity:
                    nc.vector.tensor_scalar(dst[:, d], ps[:], bias[:, d:d + 1],
                                            None, ALU.add)
                else:
                    nc.scalar.activation(dst[:, d], ps[:], func,
                                         bias=bias[:, d:d + 1])

        def linear_resid(dst, src, w_sb, bias, resid, nm, cin=CC, stats=None):
            for d in range(CC):
                ps = ps_mm.tile([P, T], F32, tag="mm", bufs=2, name=f"pr_{nm}{d}")
                for c in range(cin):
                    nc.tensor.matmul(ps[:], w_sb[:, c, d * P:(d + 1) * P], src[:, c],
                                     start=(c == 0), stop=(c == cin - 1))
                u = lnp.tile([P, T], F32, tag="lnu", bufs=3, name=f"t_{nm}{d}")
                nc.vector.tensor_scalar(u[:], ps[:], bias[:, d:d + 1], None, ALU.add)
                nc.vector.tensor_add(dst[:, d], u[:], resid[:, d])
                if stats is not None:
                    xb, sq, s1, s2 = stats
                    nc.vector.tensor_copy(xb[:, d], dst[:, d])
                    nc.vector.tensor_mul(sq[:, d], xb[:, d], xb[:, d])
                    nc.tensor.matmul(s1[:], ones_col[:], xb[:, d],
                                     start=(d == 0), stop=(d == CC - 1))
                    nc.tensor.matmul(s2[:], ones_col[:], sq[:, d],
                                     start=(d == 0), stop=(d == CC - 1))

        def v_tokmajor(dst, src, wv_sb, vbias, width, nm, fix=None):
            """dst [P, 4, width] token-major V (+ones cols)."""
            half = width // 2
            for tt in range(4):
                for hh in range(2):
                    ps = ps_mm.tile([P, half], F32, tag="mm", bufs=2,
                                    name=f"v_{nm}{tt}{hh}")
                    if vbias is None:
                        for c in range(CC):
                            nc.tensor.matmul(ps[:], src[:, c, tt * P:(tt + 1) * P],
                                             wv_sb[:, c, hh * half:(hh + 1) * half],
                                             start=(c == 0), stop=False)
                        m_b, sd_b = fix[1]
                        sl_ = slice(hh * half, (hh + 1) * half)
                        nc.tensor.matmul(ps[:], m_b[0:1, tt * P:(tt + 1) * P],
                                         vgwn_r[0:1, sl_], start=False, stop=False)
                        nc.tensor.matmul(ps[:], sd_b[0:1, tt * P:(tt + 1) * P],
                                         cstv_r[0:1, sl_], start=False, stop=True)
                    else:
                        for c in range(CC):
                            nc.tensor.matmul(ps[:], src[:, c, tt * P:(tt + 1) * P],
                                             wv_sb[:, c, hh * half:(hh + 1) * half],
                                             start=(c == 0), stop=False)
                        nc.tensor.matmul(ps[:], ones_row[:],
                                         vbias[:, hh * half:(hh + 1) * half],
                                         start=False, stop=True)
                    if fix is None:
                        nc.vector.tensor_copy(dst[:, tt, hh * half:(hh + 1) * half],
                                              ps[:])
                    else:
                        nc.vector.tensor_scalar(
                            dst[:, tt, hh * half:(hh + 1) * half], ps[:],
                            fix[0][:, tt:tt + 1], None, ALU.mult)

        FH = FC // 2

        def mlp_w1_load(w_dram, pool, nm, halves=(0, 1), eng=None):
            wts = []
            for half in halves:
                wt = pool.tile([P, CC, FH * P], BF16, tag="wma",
                               bufs=len(halves), name=f"wma_{nm}{half}")
                (eng or nc.scalar).dma_start(
                    wt[:], w_dram[:, half * FH * P:(half + 1) * FH * P]
                    .rearrange("(a p) n -> p a n", p=P))
                wts.append(wt)
            return wts

        def mlp_w2_load(w_dram, pool, nm, eng=None):
            wts = []
            for half in range(2):
                wt = pool.tile([P, FH, C], BF16, tag="wmb", bufs=2,
                               name=f"wmb_{nm}{half}")
                (eng or nc.scalar).dma_start(
                    wt[:], w_dram[half * FH * P:(half + 1) * FH * P, :]
                    .rearrange("(a p) n -> p a n", p=P))
                wts.append(wt)
            return wts

        def mlp_first(dst, src, wts, bias, nm):
            # dst [P, FC, T] = gelu(src @ W1 + b1)
            for fo in range(FC):
                wt = wts[fo // FH]
                f = fo % FH
                ps = ps_mm.tile([P, T], F32, tag="mm", bufs=2,
                                name=f"pm_{nm}{fo}")
                for c in range(CC):
                    nc.tensor.matmul(ps[:], wt[:, c, f * P:(f + 1) * P],
                                     src[:, c], start=(c == 0),
                                     stop=(c == CC - 1))
                nc.scalar.activation(dst[:, fo], ps[:], AF.Gelu,
                                     bias=bias[:, fo:fo + 1])

        def mlp_second(dst, src, wts, bias, resid, nm, stats=None):
            # dst [P, CC, T] = src @ W2 + b2 + resid
            for d in range(CC):
                ps = ps_mm.tile([P, T], F32, tag="mm", bufs=2, name=f"pr_{nm}{d}")
                for c in range(FC):
                    wt = wts[c // FH]
                    nc.tensor.matmul(ps[:], wt[:, c % FH, d * P:(d + 1) * P],
                                     src[:, c], start=(c == 0), stop=(c == FC - 1))
                u = lnp.tile([P, T], F32, tag="lnu", bufs=3, name=f"t_{nm}{d}")
                nc.vector.tensor_scalar(u[:], ps[:], bias[:, d:d + 1], None, ALU.add)
                nc.vector.tensor_add(dst[:, d], u[:], resid[:, d])
                if stats is not None:
                    xb, sq, s1, s2 = stats
                    nc.vector.tensor_copy(xb[:, d], dst[:, d])
                    nc.vector.tensor_mul(sq[:, d], xb[:, d], xb[:, d])
                    nc.tensor.matmul(s1[:], ones_col[:], xb[:, d],
                                     start=(d == 0), stop=(d == CC - 1))
                    nc.tensor.matmul(s2[:], ones_col[:], sq[:, d],
                                     start=(d == 0), stop=(d == CC - 1))

        def tap(nm, src):
            if KDBG and nm in dbg:
                for c in range(CC):
                    nc.gpsimd.dma_start(
                        dbg[nm][:].rearrange("(a p) n -> p a n", p=P)[:, c], src[:, c])

        # ===================== phase A =====================
        es_x = ExitStack()
        pgx = es_x.enter_context(tc.tile_pool(name="pgx", bufs=1, side="left"))
        es_kv = ExitStack()
        pgkv = es_kv.enter_context(tc.tile_pool(name="pgkv", bufs=1, side="left"))
        es_x1 = ExitStack()
        pgx1 = es_x1.enter_context(tc.tile_pool(name="pgx1", bufs=1, side="right"))
        es_vf = ExitStack()
        pgvf = es_vf.enter_context(tc.tile_pool(name="pgvf", bufs=1, side="right"))
        es_a = ExitStack()
        pga = es_a.enter_context(tc.tile_pool(name="pga", bufs=1, side="left"))

        # DMA priority: xTb + wk + wv feed the K/V GEMMs that gate the
        # AllGather trigger — they go first on the sync queue; y/z follow.
        # The fp32 x (residual path only) loads later, off the critical path.
        xTb = pga.tile([P, CC, T], BF16, tag="xTb", name="xTb_sb")
        for c in range(CC):
            q = nc.sync if c % 2 == 0 else nc.scalar
            q.dma_start(
                xTb[:, c], xTb_d[:].rearrange("(a p) n -> p a n", p=P)[:, c])
        wk = load_w66(w_k_d, "wk", dtype=F8)
        wv = pga.tile([P, CC, WVS], F8, tag="wv", bufs=1, name="wv_sb")
        nc.sync.dma_start(wv[:], w_ve_d[:].rearrange("(a p) n -> p a n", p=P))
        wq0 = load_w66(w_q_d, "wq0")
        yT = pga.tile([P, CC, T], BF16, tag="yT", name="yT_sb")
        for c in range(CC):
            nc.sync.dma_start(
                yT[:, c], yT_d[:].rearrange("(a p) n -> p a n", p=P)[:, c])
        zT = pga.tile([P, CC, T], BF16, tag="zT", name="zT_sb")
        for c in range(CC):
            nc.sync.dma_start(
                zT[:, c], zT_d[:].rearrange("(a p) n -> p a n", p=P)[:, c])

        bc, xb_lx = ln_stats(xTb, "lx")
        warm_fill(6)
        rstd_lx, mr_lx = stats_cells["lx"]
        m_lx, sd_lx = stats_cells["lx.msd"]
        m_b = statp.tile([1, T], BF16, tag="statc", bufs=2, name="mb_lx")
        nc.vector.tensor_copy(m_b[:], m_lx[:])
        sd_b = statp.tile([1, T], BF16, tag="statc", bufs=2, name="sdb_lx")
        nc.vector.tensor_copy(sd_b[:], sd_lx[:])
        # transpose rstd [1,512] -> [128,4] via 4 rank-1 matmuls (no DRAM trip)
        rT = lnp.tile([P, 8], F32, tag="rT", bufs=1, name="rT_lx")
        rp = ps_mm.tile([P, 4], F32, tag="mm", bufs=2, name="rT_ps")
        rstd_b_lx = stats_cells["lx.rb"]
        for a in range(4):
            nc.tensor.matmul(rp[:, a:a + 1], rstd_b_lx[0:1, a * P:(a + 1) * P],
                             ones_col[0:1, 0:1], start=True, stop=True)
        nc.vector.tensor_scalar(rT[:, 0:4], rp[:], 1.0 / 16.0, None, ALU.mult)
        warm_fill(6)

        # K = rstd*(x.g@Wk - m*(g@Wk) + (1/rstd)*(b@Wk+kb))  (LN folded into Wk)
        KTl = pga.tile([P, CC, T], F8, tag="KTl", name="KTl_sb")
        for d in range(CC):
            ps = ps_mm.tile([P, T], F32, tag="mm", bufs=2, name=f"pk{d}")
            for c in range(CC):
                nc.tensor.matmul(ps[:], wk[:, c, d * P:(d + 1) * P], xb_lx[:, c],
                                 start=(c == 0), stop=False)
            nc.tensor.matmul(ps[:], kgwn_r[0:1, d * P:(d + 1) * P], m_b[0:1, :],
                             start=False, stop=False)
            nc.tensor.matmul(ps[:], kcst_r[0:1, d * P:(d + 1) * P], sd_b[0:1, :],
                             start=False, stop=True)
            u = lnp.tile([P, T], F32, tag="lnu", bufs=3, name=f"ku{d}")
            nc.vector.tensor_scalar(u[:], ps[:], 1.0 / 16.0, None, ALU.mult)
            nc.vector.tensor_mul(KTl[:, d], u[:], bc[:, 0])
            nc.scalar.dma_start(
                kvKV_in[0:KT_E].rearrange("(a p n) -> p a n", p=P, n=T)[:, d],
                KTl[:, d])

        Vl = pga.tile([P, 4, WVS], F8, tag="Vl", name="Vl_sb")
        v_tokmajor(Vl, xb_lx, wv, None, WVS, "vs", fix=(rT[:, 0:4], (m_b, sd_b)))
        nc.scalar.dma_start(
            kvKV_in[KT_E:KVS].rearrange("(a p n) -> p a n", p=P, n=WVP)[:, :, 0:WVS],
            Vl[:])
        nc.gpsimd.collective_compute(
            "AllGather", ALU.bypass, replica_groups=[list(range(R))],
            ins=[kvKV_in[:]], outs=[kvKV_out[:]])
        Vfull = pgvf.tile([P, NKC, WVP], F8, tag="Vfull", name="Vfull_sb")
        for r in range(R):
            nc.gpsimd.dma_start(
                Vfull[:, 4 * r:4 * (r + 1), :],
                kvKV_out[r, KT_E:KVS].rearrange("(a p n) -> p a n", p=P, n=WVP))


        # Q feature-contiguous (LN folded into Wq), then duplicate per head
        QTf = pga.tile([P, CC, T], BF16, tag="hyz", bufs=1, name="QTf_sb")
        for d in range(CC):
            ps = ps_mm.tile([P, T], F32, tag="mm", bufs=2, name=f"pq{d}")
            for c in range(CC):
                nc.tensor.matmul(ps[:], wq0[:, c, d * P:(d + 1) * P], xb_lx[:, c],
                                 start=(c == 0), stop=False)
            nc.tensor.matmul(ps[:], sqgwn_r[0:1, d * P:(d + 1) * P], m_b[0:1, :],
                             start=False, stop=False)
            nc.tensor.matmul(ps[:], sqcst_r[0:1, d * P:(d + 1) * P], sd_b[0:1, :],
                             start=False, stop=True)
            uq = lnp.tile([P, T], F32, tag="lnu", bufs=3, name=f"qu{d}")
            nc.vector.tensor_copy(uq[:], ps[:])
            nc.vector.tensor_mul(QTf[:, d], uq[:], bc[:, 0])
        QT = pgx.tile([P, NHS, T], BF16, tag="QT", name="QT_sb")
        for h in range(NHS):
            src_lo = QTf[HDS * (h % 2):HDS * (h % 2) + HDS, h // 2, :]
            nc.sync.dma_start(QT[0:HDS, h, :], src_lo)
            nc.sync.dma_start(QT[HDS:P, h, :], src_lo)

        # cross-attention K/V from y, z (overlaps the AllGather above)
        hy = pga.tile([P, CC, T], BF16, tag="hyz", bufs=1, name="hy_sb")
        bcy, xb_ly = ln_stats(yT, "ly")
        ln_apply(yT, bcy, vec["ln1_g"], vec["ln1_b"], hy, "ly")
        KcT = pgkv.tile([HDC, NHC, T], F8, tag="KcT", name="KcT_sb")
        wcak = load_w66(w_cak_d, "wcak")
        for h in range(NHC):
            ps = ps_mm.tile([HDC, T], F32, tag="mm", bufs=2, name=f"kc{h}")
            for c in range(CC):
                nc.tensor.matmul(ps[:], wcak[:, c, HDC * h:HDC * (h + 1)], hy[:, c],
                                 start=(c == 0), stop=(c == CC - 1))
            nc.vector.tensor_copy(KcT[:, h], ps[:])

        hz = pga.tile([P, CC, T], BF16, tag="hyz", bufs=1, name="hz_sb")
        bcz, xb_lz = ln_stats(zT, "lz")
        ln_apply(zT, bcz, vec["ln1_g"], vec["ln1_b"], hz, "lz")
        wvc = pga.tile([P, CC, WVC], BF16, tag="wvc", bufs=1, name="wvc_sb")
        nc.sync.dma_start(wvc[:], w_cave_d[:].rearrange("(a p) n -> p a n", p=P))
        xT = pgx.tile([P, CC, T], F32, tag="xT", name="xT_sb")
        for c in range(CC):
            nc.sync.dma_start(
                xT[:, c], xT_d[:].rearrange("(a p) n -> p a n", p=P)[:, c])
        Vcl = pgkv.tile([P, 4, WVC], F8, tag="Vcl", name="Vcl_sb")
        v_tokmajor(Vcl, hz, wvc, vbc_sb, WVC, "vc")
        # export cross K/V + launch its AllGather (overlaps self-attn)
        nc.sync.dma_start(
            kvC_in[0:KT_E].rearrange("(a p n) -> p a n", p=P, n=T), KcT[:])
        nc.sync.dma_start(
            kvC_in[KT_E:KVC].rearrange("(a p n) -> p a n", p=P, n=WVP)[:, :, 0:WVC],
            Vcl[:])
        nc.gpsimd.collective_compute(
            "AllGather", ALU.bypass, replica_groups=[list(range(R))],
            ins=[kvC_in[:]], outs=[kvC_out[:]])
        es_a.close()
        es_kv.close()

        # ===================== phase B: self-attention =====================
        es_b = ExitStack()
        pgb = es_b.enter_context(tc.tile_pool(name="pgb", bufs=1, side="right"))
        ktp = pgb
        exp_p = pgb
        atp = pgb

        AT = atp.tile([P, CC, T], BF16, tag="at", name="AT_self")
        sc_s = float(HDS) ** -0.5
        exA_s = sc_s * LOG2E * 128.0
        NPR = NKC // 2  # 16 chunk-pairs
        for h in range(NHS):
            # packed K^T: partitions 0-63 = even chunk, 64-127 = odd chunk
            kt = ktp.tile([P, NPR, P], F8, tag="kt", bufs=2, name=f"ktS{h}")
            kq = nc.scalar if h < 2 else nc.sync
            for r in range(R):
                src = kvKV_out[r, HDS * h * T:(HDS * h + HDS) * T].rearrange(
                    "(p a b n) -> p a b n", p=HDS, a=2, b=2, n=P)
                kq.dma_start(kt[0:HDS, 2 * r:2 * r + 2, :], src[:, :, 0, :])
                kq.dma_start(kt[HDS:P, 2 * r:2 * r + 2, :], src[:, :, 1, :])
            ot = ps_ot.tile([HDS + 1, T], F32, tag="ot", bufs=2, name=f"otS{h}")

            def pv_s(b2, pair, h=h, ot=ot):
                for u in range(2):
                    j = 2 * b2 + u
                    nc.tensor.matmul(ot[:], Vfull[:, j, 65 * h:65 * h + 65],
                                     pair[u], start=(j == 0), stop=(j == NKC - 1),
                                     skip_group_check=True)

            # software pipeline: scores+exp(b2) issue while PV(b2-1) runs, so
            # the PE never waits out the ~0.7us exp latency
            pend = None
            for b2 in range(NPR):
                stA = ps_st.tile([P, T], F32, tag="st", bufs=4,
                                 name=f"sA{h}_{b2}")
                stB = ps_st.tile([P, T], F32, tag="st", bufs=4,
                                 name=f"sB{h}_{b2}")
                nc.tensor.matmul(stA[:], kt[0:HDS, b2, :], QT[0:HDS, h, :],
                                 start=True, stop=True)
                nc.tensor.matmul(stB[:], kt[HDS:P, b2, :], QT[HDS:P, h, :],
                                 start=True, stop=True)
                exA = exp_p.tile([P, T], I16, tag="ex", bufs=8, name=f"eA{h}_{b2}")
                nc.vector.tensor_scalar(exA[:], stA[:], exA_s, SCHRAU_B,
                                        ALU.mult, ALU.add)
                exB = exp_p.tile([P, T], BF16, tag="ex", bufs=8, name=f"eB{h}_{b2}")
                nc.scalar.activation(exB[:], stB[:], AF.Exp, scale=sc_s)
                if pend is not None:
                    pv_s(b2 - 1, pend)
                pend = (exA[:].bitcast(BF16), exB[:])
            pv_s(NPR - 1, pend)
            denr = statp.tile([1, T], F32, tag="stat", bufs=4, name=f"denrS{h}")
            nc.vector.tensor_copy(denr[:], ot[HDS:HDS + 1, :])
            den = statp.tile([1, T], F32, tag="stat", bufs=4, name=f"denS{h}")
            nc.vector.reciprocal_approx_fast(den[:], denr[:])
            bcd = lnp.tile([HDS, T], F32, tag="bcd", bufs=2, name=f"bcdS{h}")
            nc.gpsimd.partition_broadcast(bcd[:], den[:])
            nc.vector.tensor_mul(AT[HDS * (h % 2):HDS * (h % 2) + HDS, h // 2, :],
                                 ot[0:HDS, :], bcd[:])


        # proj + residual -> x1
        x1 = pgx1.tile([P, CC, T], F32, tag="x1", name="x1_sb")
        wpj = load_w66(w_proj_d, "wpj")
        xb1 = lnp.tile([P, CC, T], BF16, tag="lnxb", bufs=1, name="xb_l1")
        sq1 = lnp.tile([P, CC, T], BF16, tag="lnsq", bufs=1, name="sq_l1")
        s1_1 = ps_ot.tile([1, T], F32, tag="ot", bufs=2, name="s1_l1")
        s2_1 = ps_ot.tile([1, T], F32, tag="ot", bufs=2, name="s2_l1")
        linear_resid(x1, AT, wpj, vec["projb"], xT, "pj",
                     stats=(xb1, sq1, s1_1, s2_1))
        tap("d_x1", x1)
        es_x.close()
        es_b.close()
        es_vf.close()

        # ===================== phase C: MLP2 =====================
        es_pre = ExitStack()
        ppre = es_pre.enter_context(tc.tile_pool(name="ppre", bufs=1, side="left"))
        es_x1f = ExitStack()
        pgx1f = es_x1f.enter_context(tc.tile_pool(name="pgx1f", bufs=1, side="left"))
        es_c = ExitStack()
        pgc = es_c.enter_context(tc.tile_pool(name="pgc", bufs=1, side="left"))
        wts2a = mlp_w1_load(w_m2a_d, pgc, "m2a", eng=nc.sync)
        wts2b = mlp_w2_load(w_m2b_d, pgc, "m2b", eng=nc.sync)

        h2 = pgc.tile([P, CC, T], BF16, tag="h2", name="h2_sb")
        warm_fill(8)
        bc1 = ln_stats_from_sums(s1_1, s2_1, "l1")
        ln_apply(x1, bc1, vec["ln2_g"], vec["ln2_b"], h2, "l1")

        HT = pgc.tile([P, FC, T], BF16, tag="ht", name="HT2_sb")
        mlp_first(HT, h2, wts2a, vec["m2b1"], "m2a")
        x1f = pgx1f.tile([P, CC, T], F32, tag="x1f", name="x1f_sb")
        xbq = lnp.tile([P, CC, T], BF16, tag="lnxb", bufs=1, name="xb_lq")
        sqq = lnp.tile([P, CC, T], BF16, tag="lnsq", bufs=1, name="sq_lq")
        s1q = ps_ot.tile([1, T], F32, tag="ot", bufs=2, name="s1_lq")
        s2q = ps_ot.tile([1, T], F32, tag="ot", bufs=2, name="s2_lq")
        mlp_second(x1f, HT, wts2b, vec["m2b2"], x1, "m2b",
                   stats=(xbq, sqq, s1q, s2q))

        for c in range(CC):
            nc.gpsimd.dma_start(
                o_p1[:].rearrange("(a p) n -> p a n", p=P)[:, c], x1f[:, c])
        tap("d_x1f", x1f)
        es_x1.close()
        es_c.close()

        # ===================== phase D: cross-attention =====================
        es_x2 = ExitStack()
        pgx2 = es_x2.enter_context(tc.tile_pool(name="pgx2", bufs=1, side="right"))
        es_d = ExitStack()
        pgd = es_d.enter_context(tc.tile_pool(name="pgd", bufs=1, side="right"))
        ktp = pgd
        exp_p = pgd
        atp = pgd

        wcaq = load_w66(w_caq_d, "wcaq")

        # prefetch: cross-V staging + phase-E first-layer weights (overlap QcT/LN)
        Vcfull = pgd.tile([P, NKC, WVP], F8, tag="Vcfull", name="Vcfull_sb")
        for r in range(R):
            vq = (nc.gpsimd, nc.scalar, nc.sync)[r % 3]
            vq.dma_start(
                Vcfull[:, 4 * r:4 * (r + 1), :],
                kvC_out[r, KT_E:KVC].rearrange("(a p n) -> p a n", p=P, n=WVP))

        warm_fill(10)
        bcq = ln_stats_from_sums(s1q, s2q, "lq")
        m_lq, sd_lq = stats_cells["lq.msd"]
        mq_b = statp.tile([1, T], BF16, tag="statc", bufs=2, name="mb_lq")
        nc.vector.tensor_copy(mq_b[:], m_lq[:])
        sdq_b = statp.tile([1, T], BF16, tag="statc", bufs=2, name="sdb_lq")
        nc.vector.tensor_copy(sdq_b[:], sd_lq[:])

        QcT = pgd.tile([HDC, NHC, T], BF16, tag="QcT", name="QcT_sb")

        def qc_head(h):
            ps = ps_mm.tile([HDC, T], F32, tag="mm", bufs=2, name=f"qc{h}")
            for c in range(CC):
                nc.tensor.matmul(ps[:], wcaq[:, c, HDC * h:HDC * (h + 1)],
                                 xbq[:, c], start=(c == 0), stop=False)
            nc.tensor.matmul(ps[:], qgwn_r[0:1, HDC * h:HDC * (h + 1)], mq_b[0:1, :],
                             start=False, stop=False)
            nc.tensor.matmul(ps[:], qcst_r[0:1, HDC * h:HDC * (h + 1)], sdq_b[0:1, :],
                             start=False, stop=True)
            u = lnp.tile([P, T], F32, tag="lnu", bufs=3, name=f"qcu{h}")
            nc.vector.tensor_copy(u[0:HDC, :], ps[:])
            nc.vector.tensor_mul(QcT[:, h], u[0:HDC, :], bcq[0:HDC, 0])

        qc_head(0)
        qc_head(1)

        AcT = atp.tile([HDC, NHC, T], BF16, tag="atc", name="AT_cross")
        sc_c = float(HDC) ** -0.5
        exA_c = sc_c * LOG2E * 128.0
        for h in range(NHC):
            kt = ktp.tile([HDC, NKC, P], F8, tag="kt", bufs=2, name=f"ktC{h}")
            kq = nc.scalar if h < 2 else nc.sync
            for r in range(R):
                kq.dma_start(
                    kt[:, 4 * r:4 * (r + 1), :],
                    kvC_out[r, HDC * h * T:(HDC * h + HDC) * T]
                    .rearrange("(p j n) -> p j n", p=HDC, n=P))
            if h + 2 < NHC:
                qc_head(h + 2)
            qrhs = QcT[:, h, :]
            ot = ps_ot.tile([HDC + 1, T], F32, tag="ot", bufs=2, name=f"otC{h}")

            def pv_c(b2, pair, h=h, ot=ot):
                for u in range(2):
                    j = 2 * b2 + u
                    nc.tensor.matmul(ot[:], Vcfull[:, j, 97 * h:97 * h + 97],
                                     pair[u], start=(j == 0), stop=(j == NKC - 1),
                                     skip_group_check=True)

            pend = None
            for b2 in range(NKC // 2):
                stA = ps_st.tile([P, T], F32, tag="st", bufs=4, name=f"cA{h}_{b2}")
                stB = ps_st.tile([P, T], F32, tag="st", bufs=4, name=f"cB{h}_{b2}")
                nc.tensor.matmul(stA[:], kt[:, 2 * b2, :], qrhs,
                                 start=True, stop=True)
                nc.tensor.matmul(stB[:], kt[:, 2 * b2 + 1, :], qrhs,
                                 start=True, stop=True)
                exA = exp_p.tile([P, T], I16, tag="ex", bufs=8, name=f"cEA{h}_{b2}")
                nc.vector.tensor_scalar(exA[:], stA[:], exA_c, SCHRAU_B,
                                        ALU.mult, ALU.add)
                exB = exp_p.tile([P, T], BF16, tag="ex", bufs=8, name=f"cEB{h}_{b2}")
                nc.scalar.activation(exB[:], stB[:], AF.Exp, scale=sc_c)
                if pend is not None:
                    pv_c(b2 - 1, pend)
                pend = (exA[:].bitcast(BF16), exB[:])
            pv_c(NKC // 2 - 1, pend)
            denr = statp.tile([1, T], F32, tag="stat", bufs=4, name=f"denrC{h}")
            nc.vector.tensor_copy(denr[:], ot[HDC:HDC + 1, :])
            den = statp.tile([1, T], F32, tag="stat", bufs=4, name=f"denC{h}")
            nc.vector.reciprocal_approx_fast(den[:], denr[:])
            bcd = lnp.tile([HDC, T], F32, tag="bcd", bufs=2, name=f"bcdC{h}")
            nc.gpsimd.partition_broadcast(bcd[:], den[:])
            nc.vector.tensor_mul(AcT[:, h, :], ot[0:HDC, :], bcd[:])

        # ca_o + residual -> x2
        x2 = pgx2.tile([P, CC, T], F32, tag="x2", name="x2_sb")
        wcao = pgd.tile([HDC, NHC, C], BF16, tag="wcao", name="wcao_sb")
        nc.sync.dma_start(wcao[:], w_cao_d[:].rearrange("(a p) n -> p a n", p=HDC))
        xb2 = lnp.tile([P, CC, T], BF16, tag="lnxb", bufs=1, name="xb_l2")
        sq2 = lnp.tile([P, CC, T], BF16, tag="lnsq", bufs=1, name="sq_l2")
        s1_2 = ps_ot.tile([1, T], F32, tag="ot", bufs=2, name="s1_l2")
        s2_2 = ps_ot.tile([1, T], F32, tag="ot", bufs=2, name="s2_l2")
        for d in range(CC):
            ps = ps_mm.tile([P, T], F32, tag="mm", bufs=2, name=f"cao{d}")
            for h in range(NHC):
                nc.tensor.matmul(ps[:], wcao[:, h, d * P:(d + 1) * P], AcT[:, h, :],
                                 start=(h == 0), stop=(h == NHC - 1))
            u = lnp.tile([P, T], F32, tag="lnu", bufs=3, name=f"tcao{d}")
            nc.vector.tensor_scalar(u[:], ps[:], vec["caob"][:, d:d + 1], None,
                                    ALU.add)
            nc.vector.tensor_add(x2[:, d], u[:], x1f[:, d])
            nc.vector.tensor_copy(xb2[:, d], x2[:, d])
            nc.vector.tensor_mul(sq2[:, d], xb2[:, d], xb2[:, d])
            nc.tensor.matmul(s1_2[:], ones_col[:], xb2[:, d],
                             start=(d == 0), stop=(d == CC - 1))
            nc.tensor.matmul(s2_2[:], ones_col[:], sq2[:, d],
                             start=(d == 0), stop=(d == CC - 1))
        tap("d_x2", x2)
        es_d.close()

        # ===================== phase E: MLP + pw heads =====================
        es_e = ExitStack()
        pge = es_e.enter_context(tc.tile_pool(name="pge", bufs=1, side="left"))
        wts1a = mlp_w1_load(w_m1a_d, pge, "m1a", eng=nc.sync)
        h3 = pge.tile([P, CC, T], BF16, tag="h3", name="h3_sb")
        warm_fill(8)
        bc2 = ln_stats_from_sums(s1_2, s2_2, "l2")
        ln_apply(x2, bc2, vec["ln2_g"], vec["ln2_b"], h3, "l2")

        HT1 = pge.tile([P, FC, T], BF16, tag="ht", name="HT1_sb")
        wts1b = mlp_w2_load(w_m1b_d, pge, "m1b", eng=nc.sync)
        mlp_first(HT1, h3, wts1a, vec["m1b1"], "m1a")
        p2b = pge.tile([P, CC, T], BF16, tag="p2", name="p2_sb")
        mlp_second(p2b, HT1, wts1b, vec["m1b2"], x2, "m1b")
        tap("d_p2", p2b)

        for w_d, bias, out_d, nm in ((w_pw1_d, "pw1b", o_pw1, "pw1"),
                                     (w_pw2_d, "pw2b", o_pw2, "pw2")):
            wt = load_w66(w_d, nm)
            for d in range(CC):
                ps = ps_mm.tile([P, T], F32, tag="mm", bufs=2, name=f"p_{nm}{d}")
                for c in range(CC):
                    nc.tensor.matmul(ps[:], wt[:, c, d * P:(d + 1) * P], p2b[:, c],
                                     start=(c == 0), stop=(c == CC - 1))
                u = lnp.tile([P, T], F32, tag="lnu", bufs=3, name=f"o_{nm}{d}")
                nc.scalar.activation(u[:], ps[:], AF.Gelu, bias=vec[bias][:, d:d + 1])
                nc.sync.dma_start(
                    out_d[:].rearrange("(a p) n -> p a n", p=P)[:, d], u[:])
        es_x2.close()
        es_e.close()
        es_x1f.close()
        es_pre.close()

    nc.finalize()
    return nc


def _prep_inputs(inputs):
    f32 = np.float32

    def bf(a):
        return np.ascontiguousarray(a).astype(BFNP)

    def f8(a):
        return np.ascontiguousarray(a).astype(ml_dtypes.float8_e4m3fn)

    def vec128(v, w):
        return np.ascontiguousarray(np.asarray(v, f32).reshape(w, P).T)

    x = np.asarray(inputs["x"], f32).reshape(N, C)
    y = np.asarray(inputs["y"], f32).reshape(N, C)
    z = np.asarray(inputs["z"], f32).reshape(N, C)
    xT = np.ascontiguousarray(x.T)
    yT = np.ascontiguousarray(y.T)
    zT = np.ascontiguousarray(z.T)

    qkv_w = np.asarray(inputs["qkv_w"], f32)
    qkv_b = np.asarray(inputs["qkv_b"], f32)
    g1 = np.asarray(inputs["ln1_g"], f32)
    b1 = np.asarray(inputs["ln1_b"], f32)
    w_q_raw = qkv_w[:, 0:C]
    w_q = bf(g1[:, None] * w_q_raw)
    sqgwn = -(g1 @ w_q_raw)
    sqcst = b1 @ w_q_raw + qkv_b[0:C]
    w_k_raw = qkv_w[:, C:2 * C]
    w_k = f8(16.0 * g1[:, None] * w_k_raw)
    kgwn = -16.0 * (g1 @ w_k_raw)
    kcst = 16.0 * (b1 @ w_k_raw + qkv_b[C:2 * C])
    w_v = qkv_w[:, 2 * C:3 * C]
    w_ve = np.zeros((C, WVS), f32)
    vb_e = np.zeros((1, WVS), f32)
    for h in range(NHS):
        w_ve[:, 65 * h:65 * h + 64] = w_v[:, 64 * h:64 * h + 64]
        vb_e[0, 65 * h:65 * h + 64] = qkv_b[2 * C + 64 * h:2 * C + 64 * h + 64]
        vb_e[0, 65 * h + 64] = 1.0
    vgwn = (-(g1 @ w_ve))[None, :]
    cstv = (b1 @ w_ve + vb_e[0])[None, :]
    w_ve = g1[:, None] * w_ve

    ca_v = np.asarray(inputs["ca_v_w"], f32)
    w_cave = np.zeros((C, WVC), f32)
    vbc_e = np.zeros((1, WVC), f32)
    for h in range(NHC):
        w_cave[:, 97 * h:97 * h + 96] = ca_v[:, 96 * h:96 * h + 96]
        vbc_e[0, 97 * h + 96] = 1.0

    caq = np.asarray(inputs["ca_q_w"], f32)
    qgwn = -(g1 @ caq)
    qcst = b1 @ caq
    common = {
        "w_q": w_q, "w_k": w_k, "w_ve": f8(16.0 * w_ve), "vb_e": bf(vb_e),
        "qgwn": bf(qgwn[None, :]), "qcst": bf(qcst[None, :]),
        "sqgwn": bf(sqgwn[None, :]), "sqcst": bf(sqcst[None, :]),
        "w_proj": bf(inputs["proj_w"]),
        "w_caq": bf(g1[:, None] * caq), "w_cak": bf(inputs["ca_k_w"]),
        "w_cave": bf(w_cave), "vbc_e": bf(vbc_e),
        "w_cao": bf(inputs["ca_o_w"]),
        "w_m2a": bf(inputs["mlp2_w1"]), "w_m2b": bf(inputs["mlp2_w2"]),
        "w_m1a": bf(inputs["mlp_w1"]), "w_m1b": bf(inputs["mlp_w2"]),
        "vgwn": bf(16.0 * vgwn), "cstv": bf(16.0 * cstv),
        "kgwn": bf(kgwn[None, :]), "kcst": bf(kcst[None, :]),
        "ln1_g": vec128(inputs["ln1_g"], CC), "ln1_b": vec128(inputs["ln1_b"], CC),
        "ln2_g": vec128(inputs["ln2_g"], CC), "ln2_b": vec128(inputs["ln2_b"], CC),
        "qb": vec128(qkv_b[0:C], CC), "kb": vec128(kcst, CC),
        "kgw": vec128(kgwn, CC),
        "projb": vec128(inputs["proj_b"], CC), "caob": vec128(inputs["ca_o_b"], CC),
        "m2b1": vec128(inputs["mlp2_b1"], FC), "m2b2": vec128(inputs["mlp2_b2"], CC),
        "m1b1": vec128(inputs["mlp_b1"], FC), "m1b2": vec128(inputs["mlp_b2"], CC),
    }
    for nm in ("pw1", "pw2"):
        w = np.asarray(inputs[nm + "_w"], f32)
        g = np.asarray(inputs[nm + "_bn_g"], f32)
        b = np.asarray(inputs[nm + "_bn_b"], f32)
        m = np.asarray(inputs[nm + "_bn_m"], f32)
        v = np.asarray(inputs[nm + "_bn_v"], f32)
        scale = g / np.sqrt(v + EPS)
        common["w_" + nm] = bf((w * scale[:, None]).T)
        common[nm + "b"] = vec128(b - m * scale, CC)

    in_maps = []
    for r in range(R):
        m_ = dict(common)
        m_["xT"] = np.ascontiguousarray(xT[:, r * T:(r + 1) * T])
        m_["xTb"] = np.ascontiguousarray(xT[:, r * T:(r + 1) * T]).astype(BFNP)
        m_["yT"] = np.ascontiguousarray(yT[:, r * T:(r + 1) * T]).astype(BFNP)
        m_["zT"] = np.ascontiguousarray(zT[:, r * T:(r + 1) * T]).astype(BFNP)
        in_maps.append(m_)
    return in_maps


def _run(inputs, trace=False):
    global _BUILT
    if _BUILT is None:
        _BUILT = _build()
    nc = _BUILT
    in_maps = _prep_inputs(inputs)
    res = run_bass_kernel_spmd(nc, in_maps, core_ids=list(range(R)), trace=trace)

    def gather(name):
        full = np.concatenate([res.results[r][name] for r in range(R)], axis=1)
        return np.ascontiguousarray(full.T).reshape(1, 64, 64, C)

    outs = (gather("o_p1"), gather("o_pw1"), gather("o_pw2"))
    return outs, res


def kernel(**inputs):
    outs, _ = _run(inputs, trace=False)
    return outs



# revision 39
# speedup vs baseline: 1.1233x; 1.1233x over previous
"""Trainium2 Bass kernel for nn_Block_46059229282655 (dense transformer block).

Sharding: sequence-parallel over 8 NeuronCores (512 tokens each), weights
replicated.  K/V for both attentions are AllGathered (bf16, packed).  All
activations are kept feature-major ([C_chunk=128 partitions, tokens free]) so
matmuls never need transposes; V is gathered token-major with a baked-in ones
column per head so the softmax denominator falls out of the PV matmul.
"""

import os
from contextlib import ExitStack

import numpy as np
import ml_dtypes

import concourse.bass as bass
import concourse.mybir as mybir
import concourse.tile as tile
from concourse import bacc
from concourse.bass_utils import run_bass_kernel_spmd

BFNP = ml_dtypes.bfloat16
F32 = mybir.dt.float32
BF16 = mybir.dt.bfloat16
F8 = mybir.dt.float8e4
I16 = mybir.dt.int16
I8 = mybir.dt.int8
DROW = mybir.MatmulPerfMode.DoubleRow
AF = mybir.ActivationFunctionType
ALU = mybir.AluOpType

# Schraudolph fast-exp constants: exp(x*s) ~= bitcast_bf16(int16(x*(s*log2e*128) + B))
LOG2E = 1.4426950408889634
SCHRAU_B = 127.0 * 128.0 - 8.8
SCHRAU_B8 = 7.0 * 8.0 - 0.55

R = 8            # cores
P = 128          # partitions
T = 512          # tokens per core
N = R * T        # 4096 tokens
C = 768
CC = C // P      # 6 channel chunks
NHS, HDS = 12, 64
NHC, HDC = 8, 96
FF = 3072
FC = FF // P     # 24
NKC = N // P     # 32 key chunks
WVS = NHS * (HDS + 1)   # 780
WVC = NHC * (HDC + 1)   # 776
WVP = 784               # padded V row (DoubleRow chunk stride must be %16)
KT_E = C * T            # 393216 elements of a K^T block
VS_E = T * WVP
VC_E = T * WVP
KVS = KT_E + VS_E
KVC = KT_E + VC_E
EPS = 1e-5

KDBG = bool(os.environ.get("KDBG"))

_BUILT = None


def _build():
    nc = bacc.Bacc(None, target_bir_lowering=False, debug=False)
    dt = mybir.dt

    # ---------------- I/O ----------------
    xT_d = nc.dram_tensor("xT", [C, T], F32, kind="ExternalInput")
    xTb_d = nc.dram_tensor("xTb", [C, T], BF16, kind="ExternalInput")
    yT_d = nc.dram_tensor("yT", [C, T], BF16, kind="ExternalInput")
    zT_d = nc.dram_tensor("zT", [C, T], BF16, kind="ExternalInput")

    w_q_d = nc.dram_tensor("w_q", [C, C], BF16, kind="ExternalInput")
    w_k_d = nc.dram_tensor("w_k", [C, C], F8, kind="ExternalInput")
    w_ve_d = nc.dram_tensor("w_ve", [C, WVS], F8, kind="ExternalInput")
    vb_e_d = nc.dram_tensor("vb_e", [1, WVS], BF16, kind="ExternalInput")
    vgwn_d = nc.dram_tensor("vgwn", [1, WVS], BF16, kind="ExternalInput")
    cstv_d = nc.dram_tensor("cstv", [1, WVS], BF16, kind="ExternalInput")
    kgwn_d = nc.dram_tensor("kgwn", [1, C], BF16, kind="ExternalInput")
    kcst_d = nc.dram_tensor("kcst", [1, C], BF16, kind="ExternalInput")
    qgwn_d = nc.dram_tensor("qgwn", [1, C], BF16, kind="ExternalInput")
    qcst_d = nc.dram_tensor("qcst", [1, C], BF16, kind="ExternalInput")
    sqgwn_d = nc.dram_tensor("sqgwn", [1, C], BF16, kind="ExternalInput")
    sqcst_d = nc.dram_tensor("sqcst", [1, C], BF16, kind="ExternalInput")
    w_proj_d = nc.dram_tensor("w_proj", [C, C], BF16, kind="ExternalInput")
    w_caq_d = nc.dram_tensor("w_caq", [C, C], BF16, kind="ExternalInput")
    w_cak_d = nc.dram_tensor("w_cak", [C, C], BF16, kind="ExternalInput")
    w_cave_d = nc.dram_tensor("w_cave", [C, WVC], BF16, kind="ExternalInput")
    vbc_e_d = nc.dram_tensor("vbc_e", [1, WVC], BF16, kind="ExternalInput")
    w_cao_d = nc.dram_tensor("w_cao", [C, C], BF16, kind="ExternalInput")
    w_m2a_d = nc.dram_tensor("w_m2a", [C, FF], BF16, kind="ExternalInput")
    w_m2b_d = nc.dram_tensor("w_m2b", [FF, C], BF16, kind="ExternalInput")
    w_m1a_d = nc.dram_tensor("w_m1a", [C, FF], BF16, kind="ExternalInput")
    w_m1b_d = nc.dram_tensor("w_m1b", [FF, C], BF16, kind="ExternalInput")
    w_pw1_d = nc.dram_tensor("w_pw1", [C, C], BF16, kind="ExternalInput")
    w_pw2_d = nc.dram_tensor("w_pw2", [C, C], BF16, kind="ExternalInput")

    # [parts, k] fp32 vectors (host pre-reshaped (k,parts)->T)
    vec_specs = {
        "ln1_g": (P, CC), "ln1_b": (P, CC), "ln2_g": (P, CC), "ln2_b": (P, CC),
        "qb": (P, CC), "kb": (P, CC), "kgw": (P, CC), "projb": (P, CC),
        "caob": (P, CC),
        "m2b1": (P, FC), "m2b2": (P, CC), "m1b1": (P, FC), "m1b2": (P, CC),
        "pw1b": (P, CC), "pw2b": (P, CC),
    }
    vec_d = {k: nc.dram_tensor(k, list(s), F32, kind="ExternalInput")
             for k, s in vec_specs.items()}

    o_p1 = nc.dram_tensor("o_p1", [C, T], F32, kind="ExternalOutput")
    o_pw1 = nc.dram_tensor("o_pw1", [C, T], F32, kind="ExternalOutput")
    o_pw2 = nc.dram_tensor("o_pw2", [C, T], F32, kind="ExternalOutput")

    kvKV_in = nc.dram_tensor("kvKV_in", [KVS], F8)
    kvKV_out = nc.dram_tensor("kvKV_out", [R, KVS], F8, addr_space="Shared")
    rs_scr = nc.dram_tensor("rs_scr", [T], F32)
    kvC_in = nc.dram_tensor("kvC_in", [KVC], F8)
    kvC_out = nc.dram_tensor("kvC_out", [R, KVC], F8, addr_space="Shared")

    dbg = {}
    if KDBG:
        for nm in ("d_x1", "d_x1f", "d_x2", "d_p2", "d_at", "d_h1"):
            dbg[nm] = nc.dram_tensor(nm, [C, T], F32, kind="ExternalOutput")

    with tile.TileContext(nc) as tc, ExitStack() as top:
        # ------------- global pools -------------
        cpool = top.enter_context(tc.tile_pool(name="consts", bufs=1))
        statp = top.enter_context(tc.tile_pool(name="statp", bufs=1))
        lnp = top.enter_context(tc.tile_pool(name="lnp", bufs=1))
        w66p = top.enter_context(tc.tile_pool(name="w66p", bufs=1))
        ps_st = top.enter_context(tc.tile_pool(name="ps_st", bufs=1, space="PSUM"))
        ps_ot = top.enter_context(tc.tile_pool(name="ps_ot", bufs=1, space="PSUM"))
        ps_mm = top.enter_context(tc.tile_pool(name="ps_mm", bufs=1, space="PSUM"))

        # ------------- constants -------------
        vec_specs["qb"] = (P, CC)
        vcc_names = [k for k, s in vec_specs.items() if s == (P, CC)]
        vfc_names = [k for k, s in vec_specs.items() if s == (P, FC)]
        vcc_t = cpool.tile([P, len(vcc_names) * CC], F32, tag="vcc", name="vcc_t")
        vfc_t = cpool.tile([P, len(vfc_names) * FC], F32, tag="vfc", name="vfc_t")
        vec = {}
        for i, k in enumerate(vcc_names):
            nc.gpsimd.dma_start(vcc_t[:, i * CC:(i + 1) * CC], vec_d[k][:])
            vec[k] = vcc_t[:, i * CC:(i + 1) * CC]
        for i, k in enumerate(vfc_names):
            nc.gpsimd.dma_start(vfc_t[:, i * FC:(i + 1) * FC], vec_d[k][:])
            vec[k] = vfc_t[:, i * FC:(i + 1) * FC]
        vbc_sb = cpool.tile([1, WVC], BF16, tag="vbc", name="vbc_sb")
        nc.gpsimd.dma_start(vbc_sb[:], vbc_e_d[:])
        ones_col = cpool.tile([P, 1], BF16, tag="oc", name="ones_col")
        nc.vector.memset(ones_col[:], 1.0)
        ones_row = cpool.tile([1, P], BF16, tag="or", name="ones_row")
        nc.vector.memset(ones_row[:], 1.0)
        eps_t = cpool.tile([1, 1], F32, tag="eps", name="eps_t")
        nc.vector.memset(eps_t[:], float(EPS))
        # PE warmup: sustain HAM busy-window from t~1us so phase-A GEMMs run
        # at 2.4GHz instead of the 1.2GHz cold clock
        warm_t = cpool.tile([1, T], BF16, tag="wt", name="warm_t")
        nc.vector.memset(warm_t[:], 1.0)
        for i in range(48):
            wp = ps_mm.tile([1, T], F32, tag="mm", bufs=2, name=f"warm{i}")
            nc.tensor.matmul(wp[:], ones_col[0:1, 0:1], warm_t[0:1, 0:T],
                             start=True, stop=True)
        _wf = [0]

        def warm_fill(n):
            # dependency-free PE work to bridge sparse stretches so the HAM
            # clock gate never sees an idle MID window (it is slow to re-warm)
            for _ in range(n):
                i = _wf[0]
                _wf[0] += 1
                wp = ps_mm.tile([1, T], F32, tag="mm", bufs=2, name=f"wf{i}")
                nc.tensor.matmul(wp[:], ones_col[0:1, 0:1], warm_t[0:1, 0:T],
                                 start=True, stop=True)

        vgwn_r = cpool.tile([1, WVS], BF16, tag="vgr", name="vgwn_r")
        nc.gpsimd.dma_start(vgwn_r[:], vgwn_d[:])
        cstv_r = cpool.tile([1, WVS], BF16, tag="cvr", name="cstv_r")
        nc.gpsimd.dma_start(cstv_r[:], cstv_d[:])
        kgwn_r = cpool.tile([1, C], BF16, tag="kgr", name="kgwn_r")
        nc.gpsimd.dma_start(kgwn_r[:], kgwn_d[:])
        kcst_r = cpool.tile([1, C], BF16, tag="kcr", name="kcst_r")
        nc.gpsimd.dma_start(kcst_r[:], kcst_d[:])
        qgwn_r = cpool.tile([1, C], BF16, tag="qgr", name="qgwn_r")
        nc.gpsimd.dma_start(qgwn_r[:], qgwn_d[:])
        qcst_r = cpool.tile([1, C], BF16, tag="qcr", name="qcst_r")
        nc.gpsimd.dma_start(qcst_r[:], qcst_d[:])
        sqgwn_r = cpool.tile([1, C], BF16, tag="sqg", name="sqgwn_r")
        nc.gpsimd.dma_start(sqgwn_r[:], sqgwn_d[:])
        sqcst_r = cpool.tile([1, C], BF16, tag="sqc", name="sqcst_r")
        nc.gpsimd.dma_start(sqcst_r[:], sqcst_d[:])

        # ------------- helpers -------------
        def ln_stats(src, nm, fill=False):
            """src: [P, CC, T] fp32 SBUF. Returns psum broadcasts (rstd_b, mrstd_b)."""
            if src.dtype == BF16:
                xb = src
            else:
                xb = lnp.tile([P, CC, T], BF16, tag="lnxb", bufs=1, name=f"xb_{nm}")
            sq = lnp.tile([P, CC, T], BF16, tag="lnsq", bufs=1, name=f"sq_{nm}")
            for c in range(CC):
                if xb is not src:
                    nc.vector.tensor_copy(xb[:, c], src[:, c])
                nc.vector.tensor_mul(sq[:, c], xb[:, c], xb[:, c])
            s1 = ps_mm.tile([1, T], F32, tag="mm", bufs=2, name=f"s1_{nm}")
            for c in range(CC):
                nc.tensor.matmul(s1[:], ones_col[:], xb[:, c],
                                 start=(c == 0), stop=(c == CC - 1))
            if fill:
                warm_fill(4)
            s2 = ps_mm.tile([1, T], F32, tag="mm", bufs=2, name=f"s2_{nm}")
            for c in range(CC):
                nc.tensor.matmul(s2[:], ones_col[:], sq[:, c],
                                 start=(c == 0), stop=(c == CC - 1))
            return ln_stats_from_sums(s1, s2, nm), xb

        stats_cells = {}

        def ln_stats_from_sums(s1, s2, nm):
            m = statp.tile([1, T], F32, tag="stat", bufs=4, name=f"m_{nm}")
            nc.vector.tensor_scalar(m[:], s1[:], 1.0 / C, None, ALU.mult)
            ex2 = statp.tile([1, T], F32, tag="stat", bufs=4, name=f"e2_{nm}")
            nc.vector.tensor_scalar(ex2[:], s2[:], 1.0 / C, None, ALU.mult)
            msq = statp.tile([1, T], F32, tag="stat", bufs=4, name=f"ms_{nm}")
            nc.vector.tensor_mul(msq[:], m[:], m[:])
            var = statp.tile([1, T], F32, tag="stat", bufs=4, name=f"va_{nm}")
            nc.vector.tensor_sub(var[:], ex2[:], msq[:])
            sd = statp.tile([1, T], F32, tag="stat", bufs=4, name=f"sd_{nm}")
            nc.scalar.activation(sd[:], var[:], AF.Sqrt, bias=eps_t[:])
            stats_cells[nm + ".msd"] = (m, sd)
            rstd = statp.tile([1, T], F32, tag="stat", bufs=4, name=f"rs_{nm}")
            nc.vector.reciprocal_approx_fast(rstd[:], sd[:])
            mr = statp.tile([1, T], F32, tag="stat", bufs=4, name=f"mr_{nm}")
            nc.vector.tensor_mul(mr[:], m[:], rstd[:])
            stats_cells[nm] = (rstd, mr)
            rstd_b = statp.tile([1, T], BF16, tag="statb", bufs=2, name=f"rb_{nm}")
            nc.vector.tensor_copy(rstd_b[:], rstd[:])
            stats_cells[nm + ".rb"] = rstd_b
            mr_b = statp.tile([1, T], BF16, tag="statb", bufs=2, name=f"mb_{nm}")
            nc.vector.tensor_copy(mr_b[:], mr[:])
            # broadcast to all partitions via PE, then evacuate to SBUF so no
            # PSUM bank stays pinned across the phase
            bc_sb = lnp.tile([P, 2, T], F32, tag="bcsb", bufs=2, name=f"bcs_{nm}")
            for i, v in enumerate((rstd_b, mr_b)):
                bp = ps_mm.tile([P, T], F32, tag="mm", bufs=2, name=f"bp_{nm}{i}")
                nc.tensor.matmul(bp[:], ones_row[:], v[:], start=True, stop=True)
                nc.vector.tensor_copy(bc_sb[:, i], bp[:])
            return bc_sb

        def ln_apply(src, bc, g, b, dst, nm, eng=None):
            """dst[:, c] = ((src*rstd) - m*rstd) * g + b, bf16 out."""
            e = eng or nc.vector
            for c in range(CC):
                u = lnp.tile([P, T], F32, tag="lnu", bufs=3, name=f"u_{nm}{c}")
                e.tensor_mul(u[:], src[:, c], bc[:, 0])
                e.tensor_sub(u[:], u[:], bc[:, 1])
                e.tensor_scalar(dst[:, c], u[:], g[:, c:c + 1], b[:, c:c + 1],
                                ALU.mult, ALU.add)

        def load_w66(dram, nm, pool=None, htag="w66", parts=P, hdim=CC,
                     eng=None, dtype=BF16):
            wp = pool or w66p
            ap = dram if isinstance(dram, bass.AP) else dram[:]
            wt = wp.tile([parts, hdim, ap.shape[-1]], dtype, tag=htag, bufs=2,
                         name=f"w_{nm}")
            (eng or nc.sync).dma_start(
                wt[:], ap.rearrange("(a p) n -> p a n", p=parts))
            return wt

        def linear_fm(dst, src, w_sb, bias, func, nm, cin=CC, dout=CC):
            """dst [P, dout, T] <- act(W^T @ src + bias); w_sb [P, cin, dout*128]."""
            for d in range(dout):
                ps = ps_mm.tile([P, T], F32, tag="mm", bufs=2, name=f"p_{nm}{d}")
                for c in range(cin):
                    nc.tensor.matmul(ps[:], w_sb[:, c, d * P:(d + 1) * P], src[:, c],
                                     start=(c == 0), stop=(c == cin - 1))
                if func is None:
                    nc.vector.tensor_copy(dst[:, d], ps[:])
                elif func is AF.Identity:
                    nc.vector.tensor_scalar(dst[:, d], ps[:], bias[:, d:d + 1],
                                            None, ALU.add)
                else:
                    nc.scalar.activation(dst[:, d], ps[:], func,
                                         bias=bias[:, d:d + 1])

        def linear_resid(dst, src, w_sb, bias, resid, nm, cin=CC, stats=None):
            for d in range(CC):
                ps = ps_mm.tile([P, T], F32, tag="mm", bufs=2, name=f"pr_{nm}{d}")
                for c in range(cin):
                    nc.tensor.matmul(ps[:], w_sb[:, c, d * P:(d + 1) * P], src[:, c],
                                     start=(c == 0), stop=(c == cin - 1))
                u = lnp.tile([P, T], F32, tag="lnu", bufs=3, name=f"t_{nm}{d}")
                nc.vector.tensor_scalar(u[:], ps[:], bias[:, d:d + 1], None, ALU.add)
                nc.vector.tensor_add(dst[:, d], u[:], resid[:, d])
                if stats is not None:
                    xb, sq, s1, s2 = stats
                    nc.vector.tensor_copy(xb[:, d], dst[:, d])
                    nc.vector.tensor_mul(sq[:, d], xb[:, d], xb[:, d])
                    nc.tensor.matmul(s1[:], ones_col[:], xb[:, d],
                                     start=(d == 0), stop=(d == CC - 1))
                    nc.tensor.matmul(s2[:], ones_col[:], sq[:, d],
                                     start=(d == 0), stop=(d == CC - 1))

        def v_tokmajor(dst, src, wv_sb, vbias, width, nm, fix=None):
            """dst [P, 4, width] token-major V (+ones cols)."""
            half = width // 2
            for tt in range(4):
                for hh in range(2):
                    ps = ps_mm.tile([P, half], F32, tag="mm", bufs=2,
                                    name=f"v_{nm}{tt}{hh}")
                    if vbias is None:
                        for c in range(CC):
                            nc.tensor.matmul(ps[:], src[:, c, tt * P:(tt + 1) * P],
                                             wv_sb[:, c, hh * half:(hh + 1) * half],
                                             start=(c == 0), stop=False)
                        m_b, sd_b = fix[1]
                        sl_ = slice(hh * half, (hh + 1) * half)
                        nc.tensor.matmul(ps[:], m_b[0:1, tt * P:(tt + 1) * P],
                                         vgwn_r[0:1, sl_], start=False, stop=False)
                        nc.tensor.matmul(ps[:], sd_b[0:1, tt * P:(tt + 1) * P],
                                         cstv_r[0:1, sl_], start=False, stop=True)
                    else:
                        for c in range(CC):
                            nc.tensor.matmul(ps[:], src[:, c, tt * P:(tt + 1) * P],
                                             wv_sb[:, c, hh * half:(hh + 1) * half],
                                             start=(c == 0), stop=False)
                        nc.tensor.matmul(ps[:], ones_row[:],
                                         vbias[:, hh * half:(hh + 1) * half],
                                         start=False, stop=True)
                    if fix is None:
                        nc.vector.tensor_copy(dst[:, tt, hh * half:(hh + 1) * half],
                                              ps[:])
                    else:
                        nc.vector.tensor_scalar(
                            dst[:, tt, hh * half:(hh + 1) * half], ps[:],
                            fix[0][:, tt:tt + 1], None, ALU.mult)

        FH = FC // 2

        def mlp_w1_load(w_dram, pool, nm, halves=(0, 1), eng=None):
            wts = []
            for half in halves:
                wt = pool.tile([P, CC, FH * P], BF16, tag="wma",
                               bufs=len(halves), name=f"wma_{nm}{half}")
                (eng or nc.scalar).dma_start(
                    wt[:], w_dram[:, half * FH * P:(half + 1) * FH * P]
                    .rearrange("(a p) n -> p a n", p=P))
                wts.append(wt)
            return wts

        def mlp_w2_load(w_dram, pool, nm, eng=None):
            wts = []
            for half in range(2):
                wt = pool.tile([P, FH, C], BF16, tag="wmb", bufs=2,
                               name=f"wmb_{nm}{half}")
                (eng or nc.scalar).dma_start(
                    wt[:], w_dram[half * FH * P:(half + 1) * FH * P, :]
                    .rearrange("(a p) n -> p a n", p=P))
                wts.append(wt)
            return wts

        def mlp_first(dst, src, wts, bias, nm):
            # dst [P, FC, T] = gelu(src @ W1 + b1)
            for fo in range(FC):
                wt = wts[fo // FH]
                f = fo % FH
                ps = ps_mm.tile([P, T], F32, tag="mm", bufs=2,
                                name=f"pm_{nm}{fo}")
                for c in range(CC):
                    nc.tensor.matmul(ps[:], wt[:, c, f * P:(f + 1) * P],
                                     src[:, c], start=(c == 0),
                                     stop=(c == CC - 1))
                nc.scalar.activation(dst[:, fo], ps[:], AF.Gelu,
                                     bias=bias[:, fo:fo + 1])

        def mlp_second(dst, src, wts, bias, resid, nm, stats=None):
            # dst [P, CC, T] = src @ W2 + b2 + resid
            for d in range(CC):
                ps = ps_mm.tile([P, T], F32, tag="mm", bufs=2, name=f"pr_{nm}{d}")
                for c in range(FC):
                    wt = wts[c // FH]
                    nc.tensor.matmul(ps[:], wt[:, c % FH, d * P:(d + 1) * P],
                                     src[:, c], start=(c == 0), stop=(c == FC - 1))
                u = lnp.tile([P, T], F32, tag="lnu", bufs=3, name=f"t_{nm}{d}")
                nc.vector.tensor_scalar(u[:], ps[:], bias[:, d:d + 1], None, ALU.add)
                nc.vector.tensor_add(dst[:, d], u[:], resid[:, d])
                if stats is not None:
                    xb, sq, s1, s2 = stats
                    nc.vector.tensor_copy(xb[:, d], dst[:, d])
                    nc.vector.tensor_mul(sq[:, d], xb[:, d], xb[:, d])
                    nc.tensor.matmul(s1[:], ones_col[:], xb[:, d],
                                     start=(d == 0), stop=(d == CC - 1))
                    nc.tensor.matmul(s2[:], ones_col[:], sq[:, d],
                                     start=(d == 0), stop=(d == CC - 1))

        def tap(nm, src):
            if KDBG and nm in dbg:
                for c in range(CC):
                    nc.gpsimd.dma_start(
                        dbg[nm][:].rearrange("(a p) n -> p a n", p=P)[:, c], src[:, c])

        # ===================== phase A =====================
        es_x = ExitStack()
        pgx = es_x.enter_context(tc.tile_pool(name="pgx", bufs=1, side="left"))
        es_kv = ExitStack()
        pgkv = es_kv.enter_context(tc.tile_pool(name="pgkv", bufs=1, side="left"))
        es_x1 = ExitStack()
        pgx1 = es_x1.enter_context(tc.tile_pool(name="pgx1", bufs=1, side="right"))
        es_vf = ExitStack()
        pgvf = es_vf.enter_context(tc.tile_pool(name="pgvf", bufs=1, side="right"))
        es_a = ExitStack()
        pga = es_a.enter_context(tc.tile_pool(name="pga", bufs=1, side="left"))

        # DMA priority: xTb + wk + wv feed the K/V GEMMs that gate the
        # AllGather trigger — they go first on the sync queue; y/z follow.
        # The fp32 x (residual path only) loads later, off the critical path.
        xTb = pga.tile([P, CC, T], BF16, tag="xTb", name="xTb_sb")
        for c in range(CC):
            q = nc.sync if c % 2 == 0 else nc.scalar
            q.dma_start(
                xTb[:, c], xTb_d[:].rearrange("(a p) n -> p a n", p=P)[:, c])
        wk = load_w66(w_k_d, "wk", dtype=F8)
        wv = pga.tile([P, CC, WVS], F8, tag="wv", bufs=1, name="wv_sb")
        nc.sync.dma_start(wv[:], w_ve_d[:].rearrange("(a p) n -> p a n", p=P))
        wq0 = load_w66(w_q_d, "wq0")
        yT = pga.tile([P, CC, T], BF16, tag="yT", name="yT_sb")
        for c in range(CC):
            nc.sync.dma_start(
                yT[:, c], yT_d[:].rearrange("(a p) n -> p a n", p=P)[:, c])
        zT = pga.tile([P, CC, T], BF16, tag="zT", name="zT_sb")
        for c in range(CC):
            nc.sync.dma_start(
                zT[:, c], zT_d[:].rearrange("(a p) n -> p a n", p=P)[:, c])

        bc, xb_lx = ln_stats(xTb, "lx", fill=True)
        warm_fill(6)
        rstd_lx, mr_lx = stats_cells["lx"]
        m_lx, sd_lx = stats_cells["lx.msd"]
        m_b = statp.tile([1, T], BF16, tag="statc", bufs=2, name="mb_lx")
        nc.vector.tensor_copy(m_b[:], m_lx[:])
        sd_b = statp.tile([1, T], BF16, tag="statc", bufs=2, name="sdb_lx")
        nc.vector.tensor_copy(sd_b[:], sd_lx[:])
        # transpose rstd [1,512] -> [128,4] via 4 rank-1 matmuls (no DRAM trip)
        rT = lnp.tile([P, 8], F32, tag="rT", bufs=1, name="rT_lx")
        rp = ps_mm.tile([P, 4], F32, tag="mm", bufs=2, name="rT_ps")
        rstd_b_lx = stats_cells["lx.rb"]
        for a in range(4):
            nc.tensor.matmul(rp[:, a:a + 1], rstd_b_lx[0:1, a * P:(a + 1) * P],
                             ones_col[0:1, 0:1], start=True, stop=True)
        nc.vector.tensor_scalar(rT[:, 0:4], rp[:], 1.0 / 16.0, None, ALU.mult)
        warm_fill(6)

        # K = rstd*(x.g@Wk - m*(g@Wk) + (1/rstd)*(b@Wk+kb))  (LN folded into Wk)
        KTl = pga.tile([P, CC, T], F8, tag="KTl", name="KTl_sb")
        for d in range(CC):
            ps = ps_mm.tile([P, T], F32, tag="mm", bufs=2, name=f"pk{d}")
            for c in range(CC):
                nc.tensor.matmul(ps[:], wk[:, c, d * P:(d + 1) * P], xb_lx[:, c],
                                 start=(c == 0), stop=False)
            nc.tensor.matmul(ps[:], kgwn_r[0:1, d * P:(d + 1) * P], m_b[0:1, :],
                             start=False, stop=False)
            nc.tensor.matmul(ps[:], kcst_r[0:1, d * P:(d + 1) * P], sd_b[0:1, :],
                             start=False, stop=True)
            u = lnp.tile([P, T], F32, tag="lnu", bufs=3, name=f"ku{d}")
            nc.vector.tensor_scalar(u[:], ps[:], 1.0 / 16.0, None, ALU.mult)
            nc.vector.tensor_mul(KTl[:, d], u[:], bc[:, 0])
            nc.scalar.dma_start(
                kvKV_in[0:KT_E].rearrange("(a p n) -> p a n", p=P, n=T)[:, d],
                KTl[:, d])

        Vl = pga.tile([P, 4, WVS], F8, tag="Vl", name="Vl_sb")
        v_tokmajor(Vl, xb_lx, wv, None, WVS, "vs", fix=(rT[:, 0:4], (m_b, sd_b)))
        nc.scalar.dma_start(
            kvKV_in[KT_E:KVS].rearrange("(a p n) -> p a n", p=P, n=WVP)[:, :, 0:WVS],
            Vl[:])
        nc.gpsimd.collective_compute(
            "AllGather", ALU.bypass, replica_groups=[list(range(R))],
            ins=[kvKV_in[:]], outs=[kvKV_out[:]])
        Vfull = pgvf.tile([P, NKC, WVP], F8, tag="Vfull", name="Vfull_sb")
        for r in range(R):
            nc.gpsimd.dma_start(
                Vfull[:, 4 * r:4 * (r + 1), :],
                kvKV_out[r, KT_E:KVS].rearrange("(a p n) -> p a n", p=P, n=WVP))


        # Q feature-contiguous (LN folded into Wq), then duplicate per head
        QTf = pga.tile([P, CC, T], BF16, tag="hyz", bufs=1, name="QTf_sb")
        for d in range(CC):
            ps = ps_mm.tile([P, T], F32, tag="mm", bufs=2, name=f"pq{d}")
            for c in range(CC):
                nc.tensor.matmul(ps[:], wq0[:, c, d * P:(d + 1) * P], xb_lx[:, c],
                                 start=(c == 0), stop=False)
            nc.tensor.matmul(ps[:], sqgwn_r[0:1, d * P:(d + 1) * P], m_b[0:1, :],
                             start=False, stop=False)
            nc.tensor.matmul(ps[:], sqcst_r[0:1, d * P:(d + 1) * P], sd_b[0:1, :],
                             start=False, stop=True)
            uq = lnp.tile([P, T], F32, tag="lnu", bufs=3, name=f"qu{d}")
            nc.vector.tensor_copy(uq[:], ps[:])
            nc.vector.tensor_mul(QTf[:, d], uq[:], bc[:, 0])
        QT = pgx.tile([P, NHS, T], BF16, tag="QT", name="QT_sb")
        for h in range(NHS):
            src_lo = QTf[HDS * (h % 2):HDS * (h % 2) + HDS, h // 2, :]
            nc.sync.dma_start(QT[0:HDS, h, :], src_lo)
            nc.sync.dma_start(QT[HDS:P, h, :], src_lo)

        # cross-attention K/V from y, z (overlaps the AllGather above)
        hy = pga.tile([P, CC, T], BF16, tag="hyz", bufs=1, name="hy_sb")
        bcy, xb_ly = ln_stats(yT, "ly")
        ln_apply(yT, bcy, vec["ln1_g"], vec["ln1_b"], hy, "ly")
        KcT = pgkv.tile([HDC, NHC, T], F8, tag="KcT", name="KcT_sb")
        wcak = load_w66(w_cak_d, "wcak")
        for h in range(NHC):
            ps = ps_mm.tile([HDC, T], F32, tag="mm", bufs=2, name=f"kc{h}")
            for c in range(CC):
                nc.tensor.matmul(ps[:], wcak[:, c, HDC * h:HDC * (h + 1)], hy[:, c],
                                 start=(c == 0), stop=(c == CC - 1))
            nc.vector.tensor_copy(KcT[:, h], ps[:])

        hz = pga.tile([P, CC, T], BF16, tag="hyz", bufs=1, name="hz_sb")
        bcz, xb_lz = ln_stats(zT, "lz")
        ln_apply(zT, bcz, vec["ln1_g"], vec["ln1_b"], hz, "lz")
        wvc = pga.tile([P, CC, WVC], BF16, tag="wvc", bufs=1, name="wvc_sb")
        nc.sync.dma_start(wvc[:], w_cave_d[:].rearrange("(a p) n -> p a n", p=P))
        xT = pgx.tile([P, CC, T], F32, tag="xT", name="xT_sb")
        for c in range(CC):
            nc.sync.dma_start(
                xT[:, c], xT_d[:].rearrange("(a p) n -> p a n", p=P)[:, c])
        Vcl = pgkv.tile([P, 4, WVC], F8, tag="Vcl", name="Vcl_sb")
        v_tokmajor(Vcl, hz, wvc, vbc_sb, WVC, "vc")
        # export cross K/V + launch its AllGather (overlaps self-attn)
        nc.sync.dma_start(
            kvC_in[0:KT_E].rearrange("(a p n) -> p a n", p=P, n=T), KcT[:])
        nc.sync.dma_start(
            kvC_in[KT_E:KVC].rearrange("(a p n) -> p a n", p=P, n=WVP)[:, :, 0:WVC],
            Vcl[:])
        nc.gpsimd.collective_compute(
            "AllGather", ALU.bypass, replica_groups=[list(range(R))],
            ins=[kvC_in[:]], outs=[kvC_out[:]])
        es_a.close()
        es_kv.close()

        # ===================== phase B: self-attention =====================
        es_b = ExitStack()
        pgb = es_b.enter_context(tc.tile_pool(name="pgb", bufs=1, side="right"))
        ktp = pgb
        exp_p = pgb
        atp = pgb

        AT = atp.tile([P, CC, T], BF16, tag="at", name="AT_self")
        sc_s = float(HDS) ** -0.5
        exA_s = sc_s * LOG2E * 128.0
        NPR = NKC // 2  # 16 chunk-pairs
        for h in range(NHS):
            # packed K^T: partitions 0-63 = even chunk, 64-127 = odd chunk
            kt = ktp.tile([P, NPR, P], F8, tag="kt", bufs=2, name=f"ktS{h}")
            kq = nc.scalar if h < 2 else nc.sync
            for r in range(R):
                src = kvKV_out[r, HDS * h * T:(HDS * h + HDS) * T].rearrange(
                    "(p a b n) -> p a b n", p=HDS, a=2, b=2, n=P)
                kq.dma_start(kt[0:HDS, 2 * r:2 * r + 2, :], src[:, :, 0, :])
                kq.dma_start(kt[HDS:P, 2 * r:2 * r + 2, :], src[:, :, 1, :])
            ot = ps_ot.tile([HDS + 1, T], F32, tag="ot", bufs=2, name=f"otS{h}")

            def pv_s(b2, pair, h=h, ot=ot):
                for u in range(2):
                    j = 2 * b2 + u
                    nc.tensor.matmul(ot[:], Vfull[:, j, 65 * h:65 * h + 65],
                                     pair[u], start=(j == 0), stop=(j == NKC - 1),
                                     skip_group_check=True)

            # software pipeline: scores+exp(b2) issue while PV(b2-1) runs, so
            # the PE never waits out the ~0.7us exp latency
            pend = None
            for b2 in range(NPR):
                stA = ps_st.tile([P, T], F32, tag="st", bufs=4,
                                 name=f"sA{h}_{b2}")
                stB = ps_st.tile([P, T], F32, tag="st", bufs=4,
                                 name=f"sB{h}_{b2}")
                nc.tensor.matmul(stA[:], kt[0:HDS, b2, :], QT[0:HDS, h, :],
                                 start=True, stop=True)
                nc.tensor.matmul(stB[:], kt[HDS:P, b2, :], QT[HDS:P, h, :],
                                 start=True, stop=True)
                exA = exp_p.tile([P, T], I16, tag="ex", bufs=8, name=f"eA{h}_{b2}")
                nc.vector.tensor_scalar(exA[:], stA[:], exA_s, SCHRAU_B,
                                        ALU.mult, ALU.add)
                exB = exp_p.tile([P, T], BF16, tag="ex", bufs=8, name=f"eB{h}_{b2}")
                nc.scalar.activation(exB[:], stB[:], AF.Exp, scale=sc_s)
                if pend is not None:
                    pv_s(b2 - 1, pend)
                pend = (exA[:].bitcast(BF16), exB[:])
            pv_s(NPR - 1, pend)
            denr = statp.tile([1, T], F32, tag="stat", bufs=4, name=f"denrS{h}")
            nc.vector.tensor_copy(denr[:], ot[HDS:HDS + 1, :])
            den = statp.tile([1, T], F32, tag="stat", bufs=4, name=f"denS{h}")
            nc.vector.reciprocal_approx_fast(den[:], denr[:])
            bcd = lnp.tile([HDS, T], F32, tag="bcd", bufs=2, name=f"bcdS{h}")
            nc.gpsimd.partition_broadcast(bcd[:], den[:])
            nc.vector.tensor_mul(AT[HDS * (h % 2):HDS * (h % 2) + HDS, h // 2, :],
                                 ot[0:HDS, :], bcd[:])


        # proj + residual -> x1
        x1 = pgx1.tile([P, CC, T], F32, tag="x1", name="x1_sb")
        wpj = load_w66(w_proj_d, "wpj")
        xb1 = lnp.tile([P, CC, T], BF16, tag="lnxb", bufs=1, name="xb_l1")
        sq1 = lnp.tile([P, CC, T], BF16, tag="lnsq", bufs=1, name="sq_l1")
        s1_1 = ps_ot.tile([1, T], F32, tag="ot", bufs=2, name="s1_l1")
        s2_1 = ps_ot.tile([1, T], F32, tag="ot", bufs=2, name="s2_l1")
        linear_resid(x1, AT, wpj, vec["projb"], xT, "pj",
                     stats=(xb1, sq1, s1_1, s2_1))
        warm_fill(4)
        tap("d_x1", x1)
        es_x.close()
        es_b.close()
        es_vf.close()

        # ===================== phase C: MLP2 =====================
        es_pre = ExitStack()
        ppre = es_pre.enter_context(tc.tile_pool(name="ppre", bufs=1, side="left"))
        es_x1f = ExitStack()
        pgx1f = es_x1f.enter_context(tc.tile_pool(name="pgx1f", bufs=1, side="left"))
        es_c = ExitStack()
        pgc = es_c.enter_context(tc.tile_pool(name="pgc", bufs=1, side="left"))
        wts2a = mlp_w1_load(w_m2a_d, pgc, "m2a", eng=nc.sync)
        wts2b = mlp_w2_load(w_m2b_d, pgc, "m2b", eng=nc.sync)

        h2 = pgc.tile([P, CC, T], BF16, tag="h2", name="h2_sb")
        warm_fill(8)
        bc1 = ln_stats_from_sums(s1_1, s2_1, "l1")
        ln_apply(x1, bc1, vec["ln2_g"], vec["ln2_b"], h2, "l1")

        HT = pgc.tile([P, FC, T], BF16, tag="ht", name="HT2_sb")
        mlp_first(HT, h2, wts2a, vec["m2b1"], "m2a")
        x1f = pgx1f.tile([P, CC, T], F32, tag="x1f", name="x1f_sb")
        xbq = lnp.tile([P, CC, T], BF16, tag="lnxb", bufs=1, name="xb_lq")
        sqq = lnp.tile([P, CC, T], BF16, tag="lnsq", bufs=1, name="sq_lq")
        s1q = ps_ot.tile([1, T], F32, tag="ot", bufs=2, name="s1_lq")
        s2q = ps_ot.tile([1, T], F32, tag="ot", bufs=2, name="s2_lq")
        mlp_second(x1f, HT, wts2b, vec["m2b2"], x1, "m2b",
                   stats=(xbq, sqq, s1q, s2q))

        for c in range(CC):
            nc.gpsimd.dma_start(
                o_p1[:].rearrange("(a p) n -> p a n", p=P)[:, c], x1f[:, c])
        tap("d_x1f", x1f)
        es_x1.close()
        es_c.close()

        # ===================== phase D: cross-attention =====================
        es_x2 = ExitStack()
        pgx2 = es_x2.enter_context(tc.tile_pool(name="pgx2", bufs=1, side="right"))
        es_d = ExitStack()
        pgd = es_d.enter_context(tc.tile_pool(name="pgd", bufs=1, side="right"))
        ktp = pgd
        exp_p = pgd
        atp = pgd

        wcaq = load_w66(w_caq_d, "wcaq")

        # prefetch: cross-V staging + phase-E first-layer weights (overlap QcT/LN)
        Vcfull = pgd.tile([P, NKC, WVP], F8, tag="Vcfull", name="Vcfull_sb")
        for r in range(R):
            vq = (nc.gpsimd, nc.scalar, nc.sync)[r % 3]
            vq.dma_start(
                Vcfull[:, 4 * r:4 * (r + 1), :],
                kvC_out[r, KT_E:KVC].rearrange("(a p n) -> p a n", p=P, n=WVP))

        warm_fill(10)
        bcq = ln_stats_from_sums(s1q, s2q, "lq")
        m_lq, sd_lq = stats_cells["lq.msd"]
        mq_b = statp.tile([1, T], BF16, tag="statc", bufs=2, name="mb_lq")
        nc.vector.tensor_copy(mq_b[:], m_lq[:])
        sdq_b = statp.tile([1, T], BF16, tag="statc", bufs=2, name="sdb_lq")
        nc.vector.tensor_copy(sdq_b[:], sd_lq[:])

        QcT = pgd.tile([HDC, NHC, T], BF16, tag="QcT", name="QcT_sb")

        def qc_head(h):
            ps = ps_mm.tile([HDC, T], F32, tag="mm", bufs=2, name=f"qc{h}")
            for c in range(CC):
                nc.tensor.matmul(ps[:], wcaq[:, c, HDC * h:HDC * (h + 1)],
                                 xbq[:, c], start=(c == 0), stop=False)
            nc.tensor.matmul(ps[:], qgwn_r[0:1, HDC * h:HDC * (h + 1)], mq_b[0:1, :],
                             start=False, stop=False)
            nc.tensor.matmul(ps[:], qcst_r[0:1, HDC * h:HDC * (h + 1)], sdq_b[0:1, :],
                             start=False, stop=True)
            u = lnp.tile([P, T], F32, tag="lnu", bufs=3, name=f"qcu{h}")
            nc.vector.tensor_copy(u[0:HDC, :], ps[:])
            nc.vector.tensor_mul(QcT[:, h], u[0:HDC, :], bcq[0:HDC, 0])

        qc_head(0)
        qc_head(1)

        AcT = atp.tile([HDC, NHC, T], BF16, tag="atc", name="AT_cross")
        sc_c = float(HDC) ** -0.5
        exA_c = sc_c * LOG2E * 128.0
        for h in range(NHC):
            kt = ktp.tile([HDC, NKC, P], F8, tag="kt", bufs=2, name=f"ktC{h}")
            kq = nc.scalar if h < 2 else nc.sync
            for r in range(R):
                kq.dma_start(
                    kt[:, 4 * r:4 * (r + 1), :],
                    kvC_out[r, HDC * h * T:(HDC * h + HDC) * T]
                    .rearrange("(p j n) -> p j n", p=HDC, n=P))
            if h + 2 < NHC:
                qc_head(h + 2)
            qrhs = QcT[:, h, :]
            ot = ps_ot.tile([HDC + 1, T], F32, tag="ot", bufs=2, name=f"otC{h}")

            def pv_c(b2, pair, h=h, ot=ot):
                for u in range(2):
                    j = 2 * b2 + u
                    nc.tensor.matmul(ot[:], Vcfull[:, j, 97 * h:97 * h + 97],
                                     pair[u], start=(j == 0), stop=(j == NKC - 1),
                                     skip_group_check=True)

            pend = None
            for b2 in range(NKC // 2):
                stA = ps_st.tile([P, T], F32, tag="st", bufs=4, name=f"cA{h}_{b2}")
                stB = ps_st.tile([P, T], F32, tag="st", bufs=4, name=f"cB{h}_{b2}")
                nc.tensor.matmul(stA[:], kt[:, 2 * b2, :], qrhs,
                                 start=True, stop=True)
                nc.tensor.matmul(stB[:], kt[:, 2 * b2 + 1, :], qrhs,
                                 start=True, stop=True)
                exA = exp_p.tile([P, T], I16, tag="ex", bufs=8, name=f"cEA{h}_{b2}")
                nc.vector.tensor_scalar(exA[:], stA[:], exA_c, SCHRAU_B,
                                        ALU.mult, ALU.add)
                exB = exp_p.tile([P, T], BF16, tag="ex", bufs=8, name=f"cEB{h}_{b2}")
                nc.scalar.activation(exB[:], stB[:], AF.Exp, scale=sc_c)
                if pend is not None:
                    pv_c(b2 - 1, pend)
                pend = (exA[:].bitcast(BF16), exB[:])
            pv_c(NKC // 2 - 1, pend)
            denr = statp.tile([1, T], F32, tag="stat", bufs=4, name=f"denrC{h}")
            nc.vector.tensor_copy(denr[:], ot[HDC:HDC + 1, :])
            den = statp.tile([1, T], F32, tag="stat", bufs=4, name=f"denC{h}")
            nc.vector.reciprocal_approx_fast(den[:], denr[:])
            bcd = lnp.tile([HDC, T], F32, tag="bcd", bufs=2, name=f"bcdC{h}")
            nc.gpsimd.partition_broadcast(bcd[:], den[:])
            nc.vector.tensor_mul(AcT[:, h, :], ot[0:HDC, :], bcd[:])

        # ca_o + residual -> x2
        x2 = pgx2.tile([P, CC, T], F32, tag="x2", name="x2_sb")
        wcao = pgd.tile([HDC, NHC, C], BF16, tag="wcao", name="wcao_sb")
        nc.sync.dma_start(wcao[:], w_cao_d[:].rearrange("(a p) n -> p a n", p=HDC))
        xb2 = lnp.tile([P, CC, T], BF16, tag="lnxb", bufs=1, name="xb_l2")
        sq2 = lnp.tile([P, CC, T], BF16, tag="lnsq", bufs=1, name="sq_l2")
        s1_2 = ps_ot.tile([1, T], F32, tag="ot", bufs=2, name="s1_l2")
        s2_2 = ps_ot.tile([1, T], F32, tag="ot", bufs=2, name="s2_l2")
        for d in range(CC):
            ps = ps_mm.tile([P, T], F32, tag="mm", bufs=2, name=f"cao{d}")
            for h in range(NHC):
                nc.tensor.matmul(ps[:], wcao[:, h, d * P:(d + 1) * P], AcT[:, h, :],
                                 start=(h == 0), stop=(h == NHC - 1))
            u = lnp.tile([P, T], F32, tag="lnu", bufs=3, name=f"tcao{d}")
            nc.vector.tensor_scalar(u[:], ps[:], vec["caob"][:, d:d + 1], None,
                                    ALU.add)
            nc.vector.tensor_add(x2[:, d], u[:], x1f[:, d])
            nc.vector.tensor_copy(xb2[:, d], x2[:, d])
            nc.vector.tensor_mul(sq2[:, d], xb2[:, d], xb2[:, d])
            nc.tensor.matmul(s1_2[:], ones_col[:], xb2[:, d],
                             start=(d == 0), stop=(d == CC - 1))
            nc.tensor.matmul(s2_2[:], ones_col[:], sq2[:, d],
                             start=(d == 0), stop=(d == CC - 1))
        tap("d_x2", x2)
        es_d.close()

        # ===================== phase E: MLP + pw heads =====================
        es_e = ExitStack()
        pge = es_e.enter_context(tc.tile_pool(name="pge", bufs=1, side="left"))
        wts1a = mlp_w1_load(w_m1a_d, pge, "m1a", eng=nc.sync)
        h3 = pge.tile([P, CC, T], BF16, tag="h3", name="h3_sb")
        warm_fill(8)
        bc2 = ln_stats_from_sums(s1_2, s2_2, "l2")
        ln_apply(x2, bc2, vec["ln2_g"], vec["ln2_b"], h3, "l2")

        HT1 = pge.tile([P, FC, T], BF16, tag="ht", name="HT1_sb")
        wts1b = mlp_w2_load(w_m1b_d, pge, "m1b", eng=nc.sync)
        mlp_first(HT1, h3, wts1a, vec["m1b1"], "m1a")
        p2b = pge.tile([P, CC, T], BF16, tag="p2", name="p2_sb")
        mlp_second(p2b, HT1, wts1b, vec["m1b2"], x2, "m1b")
        tap("d_p2", p2b)

        wt1 = load_w66(w_pw1_d, "pw1")
        wt2 = load_w66(w_pw2_d, "pw2")
        for d in range(CC):
            for wt, bias, out_d, nm in ((wt1, "pw1b", o_pw1, "pw1"),
                                        (wt2, "pw2b", o_pw2, "pw2")):
                ps = ps_mm.tile([P, T], F32, tag="mm", bufs=2, name=f"p_{nm}{d}")
                for c in range(CC):
                    nc.tensor.matmul(ps[:], wt[:, c, d * P:(d + 1) * P], p2b[:, c],
                                     start=(c == 0), stop=(c == CC - 1))
                u = lnp.tile([P, T], F32, tag="lnu", bufs=3, name=f"o_{nm}{d}")
                nc.scalar.activation(u[:], ps[:], AF.Gelu, bias=vec[bias][:, d:d + 1])
                q = nc.sync if nm == "pw1" else nc.gpsimd
                q.dma_start(
                    out_d[:].rearrange("(a p) n -> p a n", p=P)[:, d], u[:])
        es_x2.close()
        es_e.close()
        es_x1f.close()
        es_pre.close()

    nc.finalize()
    return nc


def _prep_inputs(inputs):
    f32 = np.float32

    def bf(a):
        return np.ascontiguousarray(a).astype(BFNP)

    def f8(a):
        return np.ascontiguousarray(a).astype(ml_dtypes.float8_e4m3fn)

    def vec128(v, w):
        return np.ascontiguousarray(np.asarray(v, f32).reshape(w, P).T)

    x = np.asarray(inputs["x"], f32).reshape(N, C)
    y = np.asarray(inputs["y"], f32).reshape(N, C)
    z = np.asarray(inputs["z"], f32).reshape(N, C)
    xT = np.ascontiguousarray(x.T)
    yT = np.ascontiguousarray(y.T)
    zT = np.ascontiguousarray(z.T)

    qkv_w = np.asarray(inputs["qkv_w"], f32)
    qkv_b = np.asarray(inputs["qkv_b"], f32)
    g1 = np.asarray(inputs["ln1_g"], f32)
    b1 = np.asarray(inputs["ln1_b"], f32)
    w_q_raw = qkv_w[:, 0:C]
    w_q = bf(g1[:, None] * w_q_raw)
    sqgwn = -(g1 @ w_q_raw)
    sqcst = b1 @ w_q_raw + qkv_b[0:C]
    w_k_raw = qkv_w[:, C:2 * C]
    w_k = f8(16.0 * g1[:, None] * w_k_raw)
    kgwn = -16.0 * (g1 @ w_k_raw)
    kcst = 16.0 * (b1 @ w_k_raw + qkv_b[C:2 * C])
    w_v = qkv_w[:, 2 * C:3 * C]
    w_ve = np.zeros((C, WVS), f32)
    vb_e = np.zeros((1, WVS), f32)
    for h in range(NHS):
        w_ve[:, 65 * h:65 * h + 64] = w_v[:, 64 * h:64 * h + 64]
        vb_e[0, 65 * h:65 * h + 64] = qkv_b[2 * C + 64 * h:2 * C + 64 * h + 64]
        vb_e[0, 65 * h + 64] = 1.0
    vgwn = (-(g1 @ w_ve))[None, :]
    cstv = (b1 @ w_ve + vb_e[0])[None, :]
    w_ve = g1[:, None] * w_ve

    ca_v = np.asarray(inputs["ca_v_w"], f32)
    w_cave = np.zeros((C, WVC), f32)
    vbc_e = np.zeros((1, WVC), f32)
    for h in range(NHC):
        w_cave[:, 97 * h:97 * h + 96] = ca_v[:, 96 * h:96 * h + 96]
        vbc_e[0, 97 * h + 96] = 1.0

    caq = np.asarray(inputs["ca_q_w"], f32)
    qgwn = -(g1 @ caq)
    qcst = b1 @ caq
    common = {
        "w_q": w_q, "w_k": w_k, "w_ve": f8(16.0 * w_ve), "vb_e": bf(vb_e),
        "qgwn": bf(qgwn[None, :]), "qcst": bf(qcst[None, :]),
        "sqgwn": bf(sqgwn[None, :]), "sqcst": bf(sqcst[None, :]),
        "w_proj": bf(inputs["proj_w"]),
        "w_caq": bf(g1[:, None] * caq), "w_cak": bf(inputs["ca_k_w"]),
        "w_cave": bf(w_cave), "vbc_e": bf(vbc_e),
        "w_cao": bf(inputs["ca_o_w"]),
        "w_m2a": bf(inputs["mlp2_w1"]), "w_m2b": bf(inputs["mlp2_w2"]),
        "w_m1a": bf(inputs["mlp_w1"]), "w_m1b": bf(inputs["mlp_w2"]),
        "vgwn": bf(16.0 * vgwn), "cstv": bf(16.0 * cstv),
        "kgwn": bf(kgwn[None, :]), "kcst": bf(kcst[None, :]),
        "ln1_g": vec128(inputs["ln1_g"], CC), "ln1_b": vec128(inputs["ln1_b"], CC),
        "ln2_g": vec128(inputs["ln2_g"], CC), "ln2_b": vec128(inputs["ln2_b"], CC),
        "qb": vec128(qkv_b[0:C], CC), "kb": vec128(kcst, CC),
        "kgw": vec128(kgwn, CC),
        "projb": vec128(inputs["proj_b"], CC), "caob": vec128(inputs["ca_o_b"], CC),
        "m2b1": vec128(inputs["mlp2_b1"], FC), "m2b2": vec128(inputs["mlp2_b2"], CC),
        "m1b1": vec128(inputs["mlp_b1"], FC), "m1b2": vec128(inputs["mlp_b2"], CC),
    }
    for nm in ("pw1", "pw2"):
        w = np.asarray(inputs[nm + "_w"], f32)
        g = np.asarray(inputs[nm + "_bn_g"], f32)
        b = np.asarray(inputs[nm + "_bn_b"], f32)
        m = np.asarray(inputs[nm + "_bn_m"], f32)
        v = np.asarray(inputs[nm + "_bn_v"], f32)
        scale = g / np.sqrt(v + EPS)
        common["w_" + nm] = bf((w * scale[:, None]).T)
        common[nm + "b"] = vec128(b - m * scale, CC)

    in_maps = []
    for r in range(R):
        m_ = dict(common)
        m_["xT"] = np.ascontiguousarray(xT[:, r * T:(r + 1) * T])
        m_["xTb"] = np.ascontiguousarray(xT[:, r * T:(r + 1) * T]).astype(BFNP)
        m_["yT"] = np.ascontiguousarray(yT[:, r * T:(r + 1) * T]).astype(BFNP)
        m_["zT"] = np.ascontiguousarray(zT[:, r * T:(r + 1) * T]).astype(BFNP)
        in_maps.append(m_)
    return in_maps


def _run(inputs, trace=False):
    global _BUILT
    if _BUILT is None:
        _BUILT = _build()
    nc = _BUILT
    in_maps = _prep_inputs(inputs)
    res = run_bass_kernel_spmd(nc, in_maps, core_ids=list(range(R)), trace=trace)

    def gather(name):
        full = np.concatenate([res.results[r][name] for r in range(R)], axis=1)
        return np.ascontiguousarray(full.T).reshape(1, 64, 64, C)

    outs = (gather("o_p1"), gather("o_pw1"), gather("o_pw2"))
    return outs, res


def kernel(**inputs):
    outs, _ = _run(inputs, trace=False)
    return outs

